# revision 37
# baseline (speedup 1.0000x reference)
"""Trainium2 Bass kernel for the SD-style spatial attention block:

    y = x + out_w @ attn(qkv(groupnorm(x))) + out_b    (per sample)

x: [4, 256, 64, 64] fp32.  GroupNorm(8 groups) -> 1x1 conv QKV (4 heads,
head_dim 32, seq = 64*64 = 4096) -> softmax attention -> 1x1 out conv + bias
+ residual.

Sharding over 8 NeuronCores: core c handles batch b = c//2 and query-half
h = c%2 (2048 of the 4096 query positions).  Each core receives the full
sample as bf16 (for GroupNorm stats and K/V over all positions) plus its
bf16 query slice, and produces y[b][:, 2048*h : 2048*(h+1)] WITHOUT the x
residual; the host adds the exact fp32 residual while gathering.

v11 changes over v10 (which was ScalarE-exp-bound at ~363us):
  - single bf16 copy of x on device; Q/K/V projections and GroupNorm stats
    all read it.  Input DMA drops to 3MB, x_kv prioritized on all 3 queues.
  - rstd via DVE Newton rsqrt (bit-hack seed + 2 iterations): no Sqrt
    activation -> the Exp table is loaded once at t=0 and never swapped.
  - K bias dropped entirely: a per-key additive S offset that is constant
    per softmax row cancels in A/D.  (Q bias kept; V bias folded in the
    finalize as before.)
  - V^T projection drains batched 4 tiles per DVE copy.
  - q-projection of chunk c+1 emitted inside chunk c (not chunk 0).
  - o_acc zeroed by start=True on each head's first PV matmul.
  - denominator broadcast by [128,32]-ones matmuls row-aligned to o_acc
    (no bsel matmul); finalize staggered across iterations in 256-col
    pieces; PE kept warm across the stats window by interleaved dummies.
"""
import sys

sys.path.insert(0, "/opt/trn_rl_repo")

import numpy as np

import concourse.bass as bass
import concourse.bacc as bacc
import concourse.tile as tile
from concourse import mybir
from concourse.bass_utils import run_bass_kernel_spmd

F32 = mybir.dt.float32
I32 = mybir.dt.int32
BF16 = mybir.dt.bfloat16
FP16 = mybir.dt.float16
AF = mybir.ActivationFunctionType
OP = mybir.AluOpType

C = 256          # input channels
HID = 128        # qkv hidden (4 heads x 32)
NH = 4
HD = 32
SEQ = 4096       # 64*64 spatial positions
HALF = 2048      # query positions per core
G = 8            # groups
EPS = 1e-5
SCALE = float(HD) ** -0.5
ESHIFT = -2.0    # constant exp shift; cancels in O/D normalization

N_IC = HALF // 512   # i-chunks per core (4)
N_JT = SEQ // 128    # j-tiles (32)
RSQRT_MAGIC = 0x5f3759df

# Schraudolph fp16 exp on the DVE: bitcast(int16(round(u))) ~= exp(s) for
# u = 1024*log2(e)*s + 15*1024 + C, s = SCALE*S + ESHIFT.  Validated on the
# real data: all-tile approx gives 1.9e-3 end-to-end rel err; we offload
# only a fraction of tiles to shave the ScalarE-bound stream pace.
LOG2E = 1.4426950408889634
SCH_A = SCALE * LOG2E * 1024.0
SCH_B = ESHIFT * LOG2E * 1024.0 + 15.0 * 1024.0 - 44.5


def sch_offload(c, t, p):
    # mid-chunk tiles only (keep DVE free at chunk boundaries where the
    # finalize chains run)
    return p == 0 and t % 4 == 2


def build_program():
    nc = bacc.Bacc()

    x_kv = nc.declare_dram_parameter("x_kv", [C, SEQ], BF16, isOutput=False)
    x_qb = nc.declare_dram_parameter("x_qb", [C, HALF], BF16, isOutput=False)
    wqkvT = nc.declare_dram_parameter("wqkvT", [C, 3 * HID], F32, isOutput=False)
    owbT = nc.declare_dram_parameter("owbT", [HID, C], BF16, isOutput=False)
    nb = nc.declare_dram_parameter("nb", [C, 1], F32, isOutput=False)
    ob = nc.declare_dram_parameter("ob", [C, 1], F32, isOutput=False)
    gsel = nc.declare_dram_parameter("gsel", [C, 128], F32, isOutput=False)
    gselTn = nc.declare_dram_parameter("gselTn", [128, C], F32, isOutput=False)
    y = nc.declare_dram_parameter("y", [C, HALF], F32, isOutput=True)

    with tile.TileContext(nc) as tc:
        import contextlib
        with contextlib.ExitStack() as ctx:
            persist = ctx.enter_context(tc.tile_pool(name="persist", bufs=1))

            # ---------------- persistent tiles ----------------
            wq_s = [persist.tile([128, 3 * HID], F32, tag=f"wqs{i}", name=f"wqs{i}") for i in range(2)]
            w2b = [persist.tile([128, 3 * HID], BF16, tag=f"w2b{i}", name=f"w2b{i}") for i in range(2)]
            ow_b = persist.tile([128, C], BF16, tag="owb", name="owb")
            gsel_t = [persist.tile([128, 128], F32, tag=f"gsel{i}", name=f"gsel{i}") for i in range(2)]
            gselTn_t = persist.tile([128, C], F32, tag="gselTn", name="gselTn")
            nb_t = [persist.tile([128, 1], F32, tag=f"nb{i}", name=f"nb{i}") for i in range(2)]
            ob_t = [persist.tile([128, 1], F32, tag=f"ob{i}", name=f"ob{i}") for i in range(2)]
            ones_h = persist.tile([128, 1], FP16, tag="ones", name="ones")
            ones32 = persist.tile([128, 32], FP16, tag="ones32", name="ones32")
            eps_t = persist.tile([128, 1], F32, tag="eps", name="eps")
            esh_t = persist.tile([128, 1], F32, tag="esh", name="esh")
            magic_t = persist.tile([128, 1], I32, tag="magic", name="magic")
            warm_t = persist.tile([128, 512], FP16, tag="warm", name="warm")
            a_t = [persist.tile([128, 1], F32, tag=f"a{i}", name=f"a{i}") for i in range(2)]
            b_t = [persist.tile([128, 1], F32, tag=f"b{i}", name=f"b{i}") for i in range(2)]
            nc.vector.memset(ones_h, 1.0)
            nc.vector.memset(ones32, 1.0)
            nc.vector.memset(eps_t, EPS)
            nc.vector.memset(esh_t, ESHIFT)
            nc.vector.memset(warm_t, 0.0)
            nc.gpsimd.memset(magic_t, RSQRT_MAGIC)

            # pin the Exp activation table NOW (only table this kernel uses;
            # all later activations are Exp/Identity which share a set)
            scrA = persist.tile([128, 1], F32, tag="scrA", name="scrA")
            nc.scalar.activation(out=scrA, in_=eps_t, func=AF.Exp)

            # ---------------- input DMA ----------------
            xkv = [persist.tile([128, SEQ], BF16, tag=f"xkv{i}", name=f"xkv{i}") for i in range(2)]
            xq = [persist.tile([128, HALF], BF16, tag=f"xq{i}", name=f"xq{i}") for i in range(2)]
            CH = 1024
            chunk_q = [
                ((0, 0), nc.sync), ((1, 0), nc.gpsimd), ((0, 1), nc.scalar),
                ((1, 1), nc.sync), ((0, 2), nc.gpsimd), ((1, 2), nc.scalar),
                ((0, 3), nc.sync), ((1, 3), nc.gpsimd),
            ]
            for (i, p), q in chunk_q:
                q.dma_start(out=xkv[i][:, CH * p:CH * (p + 1)],
                            in_=x_kv[128 * i:128 * (i + 1), CH * p:CH * (p + 1)])
            # query-half x: first 512 cols early (gates qproj0), rest later
            nc.sync.dma_start(out=xq[0][:, 0:512], in_=x_qb[0:128, 0:512])
            nc.scalar.dma_start(out=xq[1][:, 0:512], in_=x_qb[128:256, 0:512])
            for i in range(2):
                nc.gpsimd.dma_start(out=xq[i][:, 512:HALF],
                                    in_=x_qb[128 * i:128 * (i + 1), 512:HALF])
            # weights/consts ride behind x
            nc.scalar.dma_start(out=wq_s[0], in_=wqkvT[0:128, :])
            nc.scalar.dma_start(out=wq_s[1], in_=wqkvT[128:256, :])
            nc.sync.dma_start(out=ow_b, in_=owbT[:, :])
            nc.gpsimd.dma_start(out=gselTn_t, in_=gselTn[:, :])
            for i in range(2):
                nc.sync.dma_start(out=gsel_t[i], in_=gsel[128 * i:128 * (i + 1), :])
                nc.gpsimd.dma_start(out=nb_t[i], in_=nb[128 * i:128 * (i + 1), :])
                nc.gpsimd.dma_start(out=ob_t[i], in_=ob[128 * i:128 * (i + 1), :])

            kq = persist.tile([128, SEQ], BF16, tag="K", name="K")
            qq = persist.tile([128, HALF], BF16, tag="Q", name="Q")
            vt_b = persist.tile([128, SEQ], FP16, tag="VT", name="VT")
            qkvb = [persist.tile([128, 1], F32, tag=f"qkvb{m}", name=f"qkvb{m}") for m in (0, 2)]
            qkvb = {0: qkvb[0], 2: qkvb[1]}
            dp = [persist.tile([128, HALF], FP16, tag=f"dp{i}", name=f"dp{i}") for i in range(2)]
            nc.gpsimd.memset(dp[0], 0.0)
            nc.gpsimd.memset(dp[1], 0.0)

            # ---------------- GroupNorm statistics ----------------
            with tc.tile_pool(name="gn", bufs=1) as gn, \
                 tc.tile_pool(name="ps", bufs=2, space="PSUM") as ps:
                # dummy matmuls keep the PE out of its low p-state while the
                # x DMA + stats gate the real work
                dps = ps.tile([128, 2048], F32, tag="ps", name="ps")

                def warm(n):
                    for w in range(n):
                        nc.tensor.matmul(dps[0:1, 512 * (w % 2):512 * (w % 2 + 1)],
                                         ones_h, warm_t, start=True, stop=True,
                                         skip_group_check=True)
                warm(40)

                # bn_stats in chunk-arrival order
                stats = [gn.tile([128, 8, 6], F32, tag=f"st{i}", name=f"st{i}") for i in range(2)]
                stat_order = [(0, 0), (0, 1), (1, 0), (1, 1),
                              (0, 2), (0, 3), (1, 2), (1, 3),
                              (0, 4), (0, 5), (1, 4), (1, 5),
                              (0, 6), (0, 7), (1, 6), (1, 7)]
                for i, s in stat_order:
                    nc.vector.bn_stats(out=stats[i][:, s, :],
                                       in_=xkv[i][:, 512 * s:512 * (s + 1)])
                pp = [gn.tile([128, 2], F32, tag=f"pp{i}", name=f"pp{i}") for i in range(2)]
                for i in range(2):
                    mv = gn.tile([128, 2], F32, tag=f"mv{i}", name=f"mv{i}")
                    nc.vector.bn_aggr(out=mv, in_=stats[i])
                    # pp = (mean, E[x^2]) per partition
                    tmp = gn.tile([128, 1], F32, tag=f"tmp{i}", name=f"tmp{i}")
                    nc.vector.tensor_copy(pp[i][:, 0:1], mv[:, 0:1])
                    nc.vector.tensor_mul(tmp, mv[:, 0:1], mv[:, 0:1])
                    nc.vector.tensor_add(pp[i][:, 1:2], mv[:, 1:2], tmp)

                # group sums: psum[g, :] = sum over channels of group g
                gs_ps = ps.tile([128, 2048], F32, tag="ps", name="ps")
                for i in range(2):
                    nc.tensor.matmul(gs_ps[:, 0:2], gsel_t[i], pp[i],
                                     start=(i == 0), stop=(i == 1))
                # keep the PE warm across the stats-math window (in-order
                # queue: these run right after the gs matmuls)
                warm(22)
                gsb = gn.tile([128, 2], F32, tag="gsb", name="gsb")
                nc.vector.tensor_scalar_mul(gsb, gs_ps[:, 0:2], 1.0 / 32.0)
                varg = gn.tile([128, 1], F32, tag="varg", name="varg")
                tmp2 = gn.tile([128, 1], F32, tag="tmp2", name="tmp2")
                nc.vector.tensor_mul(tmp2, gsb[:, 0:1], gsb[:, 0:1])
                nc.vector.tensor_sub(varg, gsb[:, 1:2], tmp2)
                nc.vector.tensor_scalar_add(varg, varg, EPS)
                # rstd = rsqrt(varg): bit-hack seed + 2 Newton iterations (DVE
                # only -- keeps the Exp table resident on ScalarE)
                half_i = gn.tile([128, 1], I32, tag="halfi", name="halfi")
                y0b = gn.tile([128, 1], I32, tag="y0b", name="y0b")
                nc.vector.tensor_scalar(out=half_i, in0=varg.bitcast(I32),
                                        scalar1=1, scalar2=None,
                                        op0=OP.logical_shift_right)
                nc.vector.tensor_sub(y0b, magic_t, half_i)
                yk = y0b.bitcast(F32)
                rstd = gn.tile([128, 1], F32, tag="rstd", name="rstd")
                for it in range(2):
                    y2 = gn.tile([128, 1], F32, tag=f"y2_{it}", name=f"y2_{it}")
                    t_ = gn.tile([128, 1], F32, tag=f"t_{it}", name=f"t_{it}")
                    h_ = gn.tile([128, 1], F32, tag=f"h_{it}", name=f"h_{it}")
                    nxt = rstd if it == 1 else gn.tile([128, 1], F32, tag="y1", name="y1")
                    nc.vector.tensor_mul(y2, yk, yk)
                    nc.vector.tensor_mul(t_, varg, y2)
                    nc.vector.tensor_scalar(out=h_, in0=t_, scalar1=-0.5,
                                            scalar2=1.5, op0=OP.mult, op1=OP.add)
                    nc.vector.tensor_mul(nxt, yk, h_)
                    yk = nxt
                # gstats2 = (mean*rstd, rstd) per group-partition
                gstats = gn.tile([128, 2], F32, tag="gstats", name="gstats")
                nc.vector.tensor_mul(gstats[:, 0:1], gsb[:, 0:1], rstd)
                nc.vector.tensor_copy(gstats[:, 1:2], rstd)

                # broadcast to channels via nw-folded selector:
                # cs = (nw*mean*rstd, nw*rstd) ; a = cs1 ; b = nb - cs0
                for i in range(2):
                    cs_ps = ps.tile([128, 2048], F32, tag="ps", name="ps")
                    nc.tensor.matmul(cs_ps[:, 0:2], gselTn_t[:, 128 * i:128 * (i + 1)],
                                     gstats, start=True, stop=True)
                    nc.vector.tensor_copy(a_t[i], cs_ps[:, 1:2])
                    nc.vector.tensor_sub(b_t[i], nb_t[i], cs_ps[:, 0:1])

                # fold GroupNorm scale into QKV weights (bf16 out, one op)
                for i in range(2):
                    nc.vector.tensor_scalar(out=w2b[i], in0=wq_s[i],
                                            scalar1=a_t[i], scalar2=None,
                                            op0=OP.mult)
                # q bias (m=0) on the critical path; v bias (m=2) is only
                # needed at the first finalize and is emitted in the prologue.
                # k bias cancels in softmax.
                bp = ps.tile([128, 2048], F32, tag="ps", name="ps")
                for i in range(2):
                    nc.tensor.matmul(bp[:, 0:1], wq_s[i][:, 0:128],
                                     b_t[i], start=(i == 0), stop=(i == 1))
                nc.vector.tensor_copy(qkvb[0], bp[:, 0:1])

            # ---------------- attention ----------------
            with (
                tc.tile_pool(name="sgp", bufs=2, space="PSUM") as sgp,
                tc.tile_pool(name="accp", bufs=2, space="PSUM") as accp,
                tc.tile_pool(name="finp", bufs=2, space="PSUM") as finp,
                tc.tile_pool(name="apool", bufs=4) as apool,
                tc.tile_pool(name="upool", bufs=2) as upool,
                tc.tile_pool(name="fin", bufs=2) as fin,
            ):
                slots = [(c, t, p) for c in range(N_IC) for t in range(N_JT)
                         for p in range(2)]
                sg_of = {}
                acc_of = {}

                def emit_S(idx):
                    c, t, p = slots[idx]
                    sg = sgp.tile([128, 1024], F32, tag="sg", name="sg")
                    for hh in range(2):
                        h = 2 * p + hh
                        nc.tensor.matmul(
                            sg[:, 512 * hh:512 * (hh + 1)],
                            kq[32 * h:32 * (h + 1), 128 * t:128 * (t + 1)],
                            qq[32 * h:32 * (h + 1), 512 * c:512 * (c + 1)],
                            start=True, stop=True, tile_position=(32 * h, 0),
                        )
                    sg_of[idx] = sg

                def emit_qproj(icb, use_act):
                    qp = finp.tile([128, 512], F32, tag="fp", name="qp")
                    for i in range(2):
                        nc.tensor.matmul(qp, w2b[i][:, 0:HID],
                                         xq[i][:, 512 * icb:512 * (icb + 1)],
                                         start=(i == 0), stop=(i == 1))
                    dst = qq[:, 512 * icb:512 * (icb + 1)]
                    if use_act:
                        # ScalarE is idle pre-stream; Identity = in + bias
                        nc.scalar.activation(out=dst, in_=qp, func=AF.Identity,
                                             bias=qkvb[0], scale=1.0)
                    else:
                        nc.vector.tensor_scalar_add(dst, qp, qkvb[0])

                def emit_seg_K(seg):
                    # K' without bias (cancels in softmax); plain bf16 drain
                    sl = slice(512 * seg, 512 * (seg + 1))
                    kp = finp.tile([128, 512], F32, tag="fp", name="kp")
                    for i in range(2):
                        nc.tensor.matmul(kp, w2b[i][:, HID:2 * HID],
                                         xkv[i][:, sl], start=(i == 0), stop=(i == 1))
                    nc.vector.tensor_copy(kq[:, sl], kp)

                def emit_VT_group(g):
                    # V^T for j-tiles 4g..4g+3, one psum tile + one DVE drain
                    vtp = finp.tile([128, 512], F32, tag="fp", name="vtp")
                    for tt in range(4):
                        t = 4 * g + tt
                        for i in range(2):
                            nc.tensor.matmul(vtp[:, 128 * tt:128 * (tt + 1)],
                                             xkv[i][:, 128 * t:128 * (t + 1)],
                                             w2b[i][:, 2 * HID:3 * HID],
                                             start=(i == 0), stop=(i == 1))
                    nc.vector.tensor_copy(vt_b[:, 512 * g:512 * (g + 1)], vtp)

                fin_state = {}

                def finalize_d1(c):
                    # denominator matmuls, heads 0-1.  ones32 stationary
                    # broadcasts D_h over output rows 32h..32h+32, which
                    # row-aligns 1/D with o_acc's (h, d) rows -- no bsel
                    # broadcast matmul needed.
                    dcur = dp[c % 2]
                    d4 = finp.tile([128, 512], F32, tag="fp", name="d4")
                    for h in range(2):
                        nc.tensor.matmul(
                            d4[32 * h:32 * (h + 1), :], ones32,
                            dcur[:, 512 * h:512 * (h + 1)],
                            start=True, stop=True,
                            tile_position=(0, 32 * h), skip_group_check=True,
                        )
                    fin_state[c] = d4

                def finalize_d2(c):
                    # heads 2-3 + the reciprocal chain
                    dcur = dp[c % 2]
                    d4 = fin_state[c]
                    for h in range(2, NH):
                        nc.tensor.matmul(
                            d4[32 * h:32 * (h + 1), :], ones32,
                            dcur[:, 512 * h:512 * (h + 1)],
                            start=True, stop=True,
                            tile_position=(0, 32 * h), skip_group_check=True,
                        )
                    dmx = fin.tile([128, 512], F32, tag="dmx", name="dmx")
                    nc.vector.tensor_scalar_max(dmx, d4, 1e-30)
                    dr32 = fin.tile([128, 512], F32, tag="dr32", name="dr32")
                    scr = fin.tile([128, 512], F32, tag="scr", name="scr")
                    nc.vector.reciprocal_approx_accurate(out=dr32, in_=dmx,
                                                         scratch=scr)
                    drb = fin.tile([128, 512], BF16, tag="drb", name="drb")
                    nc.vector.tensor_copy(drb, dr32)
                    fin_state[c] = drb

                def finalize_piece(c, pc):
                    # normalize + out-proj + store for 256 queries; staggered
                    # across iterations to avoid a PE/DVE lump at chunk turns
                    drb = fin_state[c]
                    o_acc = acc_of[c]
                    w = 256
                    sl = slice(w * pc, w * (pc + 1))
                    o_sb = fin.tile([128, 256], F32, tag="osb", name="osb")
                    nc.vector.tensor_copy(o_sb, o_acc[:, sl])
                    on32 = fin.tile([128, 256], F32, tag="on32", name="on32")
                    on_b = fin.tile([128, 256], BF16, tag="onb", name="onb")
                    nc.vector.tensor_mul(on32, o_sb, drb[:, sl])
                    nc.vector.tensor_scalar_add(on_b, on32, qkvb[2])
                    for oc in range(2):
                        fo = finp.tile([128, 512], F32, tag="fp", name="fo")
                        nc.tensor.matmul(fo[:, 0:w],
                                         ow_b[:, 128 * oc:128 * (oc + 1)],
                                         on_b, start=True, stop=True)
                        ysb = fin.tile([128, 256], F32, tag="ysb", name="ysb")
                        nc.vector.tensor_scalar_add(ysb, fo[:, 0:w], ob_t[oc])
                        q_eng = nc.sync if oc == 0 else nc.gpsimd
                        q_eng.dma_start(
                            out=y[128 * oc:128 * (oc + 1),
                                  512 * c + w * pc:512 * c + w * (pc + 1)],
                            in_=ysb,
                        )

                def emit_PV(idx, a_t2):
                    c, t, p = slots[idx]
                    o_acc = acc_of[c]
                    last = (t == N_JT - 1 and p == 1)
                    for hh in range(2):
                        h = 2 * p + hh
                        nc.tensor.matmul(
                            o_acc[32 * h:32 * (h + 1), :],
                            vt_b[:, 128 * t + 32 * h:128 * t + 32 * (h + 1)],
                            a_t2[:, 512 * hh:512 * (hh + 1)],
                            start=(t == 0), stop=(last and hh == 1),
                            tile_position=(0, 32 * h), skip_group_check=True,
                        )

                # prologue: only seg0's K gates the first S/exp; VT group 0 is
                # needed first by PV(0), which runs after exp(0)
                emit_seg_K(0)
                emit_qproj(0, use_act=True)
                emit_S(0)
                emit_VT_group(0)
                # v bias (off critical path; first use at finalize of chunk 0)
                vbp = finp.tile([128, 512], F32, tag="fp", name="vbp")
                for i in range(2):
                    nc.tensor.matmul(vbp[:, 0:1], wq_s[i][:, 256:384],
                                     b_t[i], start=(i == 0), stop=(i == 1))
                nc.vector.tensor_copy(qkvb[2], vbp[:, 0:1])

                a_of = {}
                for idx, (c, t, p) in enumerate(slots):
                    if t == 0 and p == 0:
                        acc_of[c] = accp.tile([128, 512], F32, tag="Oacc", name="Oacc")

                    sg = sg_of.pop(idx)
                    a_t2 = apool.tile([128, 1024], FP16, tag="A", name="A")
                    a_of[idx] = a_t2
                    if sch_offload(c, t, p):
                        # DVE Schraudolph exp: u = A*S + B (fp16), then
                        # clamp+round to int16 whose BITS are the fp16 exp
                        u_t = upool.tile([128, 1024], FP16, tag="U", name="U")
                        nc.vector.tensor_scalar(out=u_t, in0=sg,
                                                scalar1=SCH_A, scalar2=SCH_B,
                                                op0=OP.mult, op1=OP.add)
                        nc.vector.tensor_scalar(out=a_t2.bitcast(mybir.dt.int16),
                                                in0=u_t,
                                                scalar1=32736.0, scalar2=0.0,
                                                op0=OP.min, op1=OP.max)
                    else:
                        nc.scalar.activation(out=a_t2, in_=sg, func=AF.Exp,
                                             scale=SCALE, bias=esh_t)
                    # S of the next half-slot goes on the PE queue BEFORE the
                    # delayed PV so the PE runs S(idx+1) first when exp(idx)
                    # completes -- ScalarE never waits on the PE.
                    if idx + 1 < len(slots):
                        emit_S(idx + 1)
                    if idx > 0:
                        emit_PV(idx - 1, a_of.pop(idx - 1))
                    if t == 0 and p == 0 and c > 0:
                        finalize_d2(c - 1)
                    if t == 0 and p == 1 and c > 0:
                        finalize_piece(c - 1, 0)
                    if t == 1 and p == 0 and c > 0:
                        finalize_piece(c - 1, 1)
                    if t == 0 and p == 0 and c + 1 < N_IC:
                        # after the delayed PV/finalize of chunk c-1 so the
                        # memset's WAR lands behind finalize's dp reads
                        nc.gpsimd.memset(dp[(c + 1) % 2], 0.0)
                    nc.vector.tensor_add(dp[c % 2][:, 1024 * p:1024 * (p + 1)],
                                         dp[c % 2][:, 1024 * p:1024 * (p + 1)],
                                         a_t2)
                    if t == N_JT - 1 and p == 1:
                        finalize_d1(c)
                    # projection spreading
                    if t == 8 and p == 0 and c + 1 < N_IC:
                        emit_qproj(c + 1, use_act=False)
                    if c == 0 and p == 1 and t % 4 == 1 and t // 4 + 1 < 8:
                        emit_seg_K(t // 4 + 1)
                    if c == 0 and p == 1 and t % 4 == 3 and t // 4 + 1 < 8:
                        emit_VT_group(t // 4 + 1)
                # tail: pipeline the last chunk's finalize in 256-col pieces
                # (o_sb copies and the D matmuls first, then per-piece
                # reciprocal chains interleaved with normalize/out-proj/DMA)
                cl = N_IC - 1
                emit_PV(n_last := len(slots) - 1, a_of.pop(n_last))
                dcur = dp[cl % 2]
                d4 = fin_state[cl]
                for h in range(2, NH):
                    nc.tensor.matmul(
                        d4[32 * h:32 * (h + 1), :], ones32,
                        dcur[:, 512 * h:512 * (h + 1)],
                        start=True, stop=True,
                        tile_position=(0, 32 * h), skip_group_check=True,
                    )
                o_sbs = []
                for pc in range(2):
                    o_sb = fin.tile([128, 256], F32, tag=f"osbt{pc}", name=f"osbt{pc}")
                    nc.vector.tensor_copy(o_sb, acc_of[cl][:, 256 * pc:256 * (pc + 1)])
                    o_sbs.append(o_sb)
                dmxs = []
                for pc in range(2):
                    # both d4 reads emitted before any finp reuse below
                    dmx = fin.tile([128, 256], F32, tag=f"dmxt{pc}", name=f"dmxt{pc}")
                    nc.vector.tensor_scalar_max(dmx, d4[:, 256 * pc:256 * (pc + 1)],
                                                1e-30)
                    dmxs.append(dmx)
                for pc in range(2):
                    dr32 = fin.tile([128, 256], F32, tag="dr32t", name="dr32t")
                    scr = fin.tile([128, 256], F32, tag="scrt", name="scrt")
                    nc.vector.reciprocal_approx_accurate(out=dr32, in_=dmxs[pc],
                                                         scratch=scr)
                    on32 = fin.tile([128, 256], F32, tag="on32", name="on32")
                    on_b = fin.tile([128, 256], BF16, tag="onb", name="onb")
                    nc.vector.tensor_mul(on32, o_sbs[pc], dr32)
                    nc.vector.tensor_scalar_add(on_b, on32, qkvb[2])
                    for oc in range(2):
                        fo = finp.tile([128, 512], F32, tag="fp", name="fo")
                        nc.tensor.matmul(fo[:, 0:256],
                                         ow_b[:, 128 * oc:128 * (oc + 1)],
                                         on_b, start=True, stop=True)
                        ysb = fin.tile([128, 256], F32, tag="ysb", name="ysb")
                        nc.vector.tensor_scalar_add(ysb, fo[:, 0:256], ob_t[oc])
                        q_eng = nc.sync if oc == 0 else nc.gpsimd
                        q_eng.dma_start(
                            out=y[128 * oc:128 * (oc + 1),
                                  512 * cl + 256 * pc:512 * cl + 256 * (pc + 1)],
                            in_=ysb,
                        )
    nc.compile()
    return nc


_NC_CACHE = {}


def _get_nc():
    if "nc" not in _NC_CACHE:
        _NC_CACHE["nc"] = build_program()
    return _NC_CACHE["nc"]


def _host_inputs(x, norm_w, norm_b, qkv_w, out_w, out_b):
    """Build the 8 per-core input maps."""
    import ml_dtypes
    x = np.asarray(x, dtype=np.float32)
    B = x.shape[0]
    xf = x.reshape(B, C, SEQ)
    xb = [np.ascontiguousarray(xf[b].astype(ml_dtypes.bfloat16)) for b in range(B)]

    wqkvT = np.ascontiguousarray(np.asarray(qkv_w, np.float32).T)      # [256, 384]
    owbT = np.ascontiguousarray(
        np.asarray(out_w, np.float32).T.astype(ml_dtypes.bfloat16))    # [128, 256]
    nw = np.asarray(norm_w, np.float32).reshape(C)
    nbv = np.asarray(norm_b, np.float32).reshape(C, 1).copy()
    obv = np.asarray(out_b, np.float32).reshape(C, 1).copy()

    gsel = np.zeros((C, 128), np.float32)
    for ch in range(C):
        gsel[ch, ch // 32] = 1.0
    # nw-folded transpose selector: gselTn[g, ch] = nw[ch] for ch in group g
    gselTn = np.zeros((128, C), np.float32)
    for ch in range(C):
        gselTn[ch // 32, ch] = nw[ch]

    in_maps = []
    for core in range(8):
        b, h = core // 2, core % 2
        in_maps.append({
            "x_kv": xb[b],
            "x_qb": np.ascontiguousarray(xb[b][:, HALF * h:HALF * (h + 1)]),
            "wqkvT": wqkvT, "owbT": owbT, "nb": nbv, "ob": obv,
            "gsel": gsel, "gselTn": gselTn,
        })
    return in_maps


def run(x, norm_w, norm_b, qkv_w, out_w, out_b, trace=False, tmpdir=None):
    """Run on 8 cores; returns (y_full, BassKernelResults)."""
    nc = _get_nc()
    in_maps = _host_inputs(x, norm_w, norm_b, qkv_w, out_w, out_b)
    res = run_bass_kernel_spmd(nc, in_maps, core_ids=list(range(8)), trace=trace,
                               tmpdir=tmpdir)
    x = np.asarray(x, dtype=np.float32)
    B = x.shape[0]
    HW_SIDE = int(np.sqrt(SEQ))
    out = np.empty((B, C, SEQ), np.float32)
    for core in range(8):
        b, h = core // 2, core % 2
        out[b][:, HALF * h:HALF * (h + 1)] = res.results[core]["y"]
    # exact fp32 residual added on host (kernel output excludes x)
    out += x.reshape(B, C, SEQ)
    return out.reshape(B, C, HW_SIDE, HW_SIDE), res


def kernel(x, norm_w, norm_b, qkv_w, out_w, out_b):
    y, _ = run(x, norm_w, norm_b, qkv_w, out_w, out_b, trace=False)
    return y


# revision 39
# speedup vs baseline: 1.2302x; 1.2302x over previous
"""Trainium2 Bass kernel for the SD-style spatial attention block:

    y = x + out_w @ attn(qkv(groupnorm(x))) + out_b    (per sample)

x: [4, 256, 64, 64] fp32.  GroupNorm(8 groups) -> 1x1 conv QKV (4 heads,
head_dim 32, seq = 64*64 = 4096) -> softmax attention -> 1x1 out conv + bias
+ residual.

Sharding over 8 NeuronCores: core c handles batch b = c//2 and query-half
h = c%2 (2048 of the 4096 query positions).  Each core receives the full
sample as bf16 (for GroupNorm stats and K/V over all positions) plus its
bf16 query slice, and produces y[b][:, 2048*h : 2048*(h+1)] WITHOUT the x
residual; the host adds the exact fp32 residual while gathering.

v11 changes over v10 (which was ScalarE-exp-bound at ~363us):
  - single bf16 copy of x on device; Q/K/V projections and GroupNorm stats
    all read it.  Input DMA drops to 3MB, x_kv prioritized on all 3 queues.
  - rstd via DVE Newton rsqrt (bit-hack seed + 2 iterations): no Sqrt
    activation -> the Exp table is loaded once at t=0 and never swapped.
  - K bias dropped entirely: a per-key additive S offset that is constant
    per softmax row cancels in A/D.  (Q bias kept; V bias folded in the
    finalize as before.)
  - V^T projection drains batched 4 tiles per DVE copy.
  - q-projection of chunk c+1 emitted inside chunk c (not chunk 0).
  - o_acc zeroed by start=True on each head's first PV matmul.
  - denominator broadcast by [128,32]-ones matmuls row-aligned to o_acc
    (no bsel matmul); finalize staggered across iterations in 256-col
    pieces; PE kept warm across the stats window by interleaved dummies.
"""
import sys

sys.path.insert(0, "/opt/trn_rl_repo")

import numpy as np

import concourse.bass as bass
import concourse.bacc as bacc
import concourse.tile as tile
from concourse import mybir
from concourse.bass_utils import run_bass_kernel_spmd

F32 = mybir.dt.float32
I32 = mybir.dt.int32
BF16 = mybir.dt.bfloat16
FP16 = mybir.dt.float16
AF = mybir.ActivationFunctionType
OP = mybir.AluOpType

C = 256          # input channels
HID = 128        # qkv hidden (4 heads x 32)
NH = 4
HD = 32
SEQ = 4096       # 64*64 spatial positions
HALF = 2048      # query positions per core
G = 8            # groups
EPS = 1e-5
SCALE = float(HD) ** -0.5
ESHIFT = -2.0    # constant exp shift; cancels in O/D normalization

N_IC = HALF // 512   # i-chunks per core (4)
N_JT = SEQ // 128    # j-tiles (32)
RSQRT_MAGIC = 0x5f3759df

# Schraudolph fp16 exp on the DVE: bitcast(int16(round(u))) ~= exp(s) for
# u = 1024*log2(e)*s + 15*1024 + C, s = SCALE*S + ESHIFT.  Validated on the
# real data: all-tile approx gives 1.9e-3 end-to-end rel err; we offload
# only a fraction of tiles to shave the ScalarE-bound stream pace.
LOG2E = 1.4426950408889634
SCH_A = SCALE * LOG2E * 1024.0
SCH_B = ESHIFT * LOG2E * 1024.0 + 15.0 * 1024.0 - 44.5


def sch_offload(c, t, p):
    # DISABLED: measured on HW, offloading whole tiles to the DVE stalls
    # the 2-deep S PSUM ring (the DVE holds the bank ~2us vs ScalarE's
    # 1.13us) and costs ~2.3us per offloaded tile.  Correctness was fine
    # (rel err 8e-4 at 1/8 offload).  Would need a 3-deep S ring.
    return False


def build_program():
    nc = bacc.Bacc()

    x_kv = nc.declare_dram_parameter("x_kv", [C, SEQ], BF16, isOutput=False)
    x_qb = nc.declare_dram_parameter("x_qb", [C, HALF], BF16, isOutput=False)
    wqkvT = nc.declare_dram_parameter("wqkvT", [C, 3 * HID], F32, isOutput=False)
    owbT = nc.declare_dram_parameter("owbT", [HID, C], BF16, isOutput=False)
    nb = nc.declare_dram_parameter("nb", [C, 1], F32, isOutput=False)
    ob = nc.declare_dram_parameter("ob", [C, 1], F32, isOutput=False)
    gsel = nc.declare_dram_parameter("gsel", [C, 128], F32, isOutput=False)
    gselTn = nc.declare_dram_parameter("gselTn", [128, C], F32, isOutput=False)
    y = nc.declare_dram_parameter("y", [C, HALF], F32, isOutput=True)

    with tile.TileContext(nc) as tc:
        import contextlib
        with contextlib.ExitStack() as ctx:
            persist = ctx.enter_context(tc.tile_pool(name="persist", bufs=1))

            # ---------------- persistent tiles ----------------
            wq_s = [persist.tile([128, 3 * HID], F32, tag=f"wqs{i}", name=f"wqs{i}") for i in range(2)]
            w2b = [persist.tile([128, 3 * HID], BF16, tag=f"w2b{i}", name=f"w2b{i}") for i in range(2)]
            ow_b = persist.tile([128, C], BF16, tag="owb", name="owb")
            gsel_t = [persist.tile([128, 128], F32, tag=f"gsel{i}", name=f"gsel{i}") for i in range(2)]
            gselTn_t = persist.tile([128, C], F32, tag="gselTn", name="gselTn")
            nb_t = [persist.tile([128, 1], F32, tag=f"nb{i}", name=f"nb{i}") for i in range(2)]
            ob_t = [persist.tile([128, 1], F32, tag=f"ob{i}", name=f"ob{i}") for i in range(2)]
            ones_h = persist.tile([128, 1], FP16, tag="ones", name="ones")
            ones32 = persist.tile([128, 32], FP16, tag="ones32", name="ones32")
            eps_t = persist.tile([128, 1], F32, tag="eps", name="eps")
            esh_t = persist.tile([128, 1], F32, tag="esh", name="esh")
            magic_t = persist.tile([128, 1], I32, tag="magic", name="magic")
            warm_t = persist.tile([128, 512], FP16, tag="warm", name="warm")
            a_t = [persist.tile([128, 1], F32, tag=f"a{i}", name=f"a{i}") for i in range(2)]
            b_t = [persist.tile([128, 1], F32, tag=f"b{i}", name=f"b{i}") for i in range(2)]
            nc.vector.memset(ones_h, 1.0)
            nc.vector.memset(ones32, 1.0)
            nc.vector.memset(eps_t, EPS)
            nc.vector.memset(esh_t, ESHIFT)
            nc.vector.memset(warm_t, 0.0)
            nc.gpsimd.memset(magic_t, RSQRT_MAGIC)

            # pin the Exp activation table NOW (only table this kernel uses;
            # all later activations are Exp/Identity which share a set)
            scrA = persist.tile([128, 1], F32, tag="scrA", name="scrA")
            nc.scalar.activation(out=scrA, in_=eps_t, func=AF.Exp)

            # ---------------- input DMA ----------------
            xkv = [persist.tile([128, SEQ], BF16, tag=f"xkv{i}", name=f"xkv{i}") for i in range(2)]
            xq = [persist.tile([128, HALF], BF16, tag=f"xq{i}", name=f"xq{i}") for i in range(2)]
            CH = 1024
            chunk_q = [
                ((0, 0), nc.sync), ((1, 0), nc.gpsimd), ((0, 1), nc.scalar),
                ((1, 1), nc.sync), ((0, 2), nc.gpsimd), ((1, 2), nc.scalar),
                ((0, 3), nc.sync), ((1, 3), nc.gpsimd),
            ]
            for (i, p), q in chunk_q:
                q.dma_start(out=xkv[i][:, CH * p:CH * (p + 1)],
                            in_=x_kv[128 * i:128 * (i + 1), CH * p:CH * (p + 1)])
            # query-half x: first 512 cols early (gates qproj0), rest later
            nc.sync.dma_start(out=xq[0][:, 0:512], in_=x_qb[0:128, 0:512])
            nc.scalar.dma_start(out=xq[1][:, 0:512], in_=x_qb[128:256, 0:512])
            for i in range(2):
                nc.gpsimd.dma_start(out=xq[i][:, 512:HALF],
                                    in_=x_qb[128 * i:128 * (i + 1), 512:HALF])
            # weights/consts ride behind x
            nc.scalar.dma_start(out=wq_s[0], in_=wqkvT[0:128, :])
            nc.scalar.dma_start(out=wq_s[1], in_=wqkvT[128:256, :])
            nc.sync.dma_start(out=ow_b, in_=owbT[:, :])
            nc.gpsimd.dma_start(out=gselTn_t, in_=gselTn[:, :])
            for i in range(2):
                nc.sync.dma_start(out=gsel_t[i], in_=gsel[128 * i:128 * (i + 1), :])
                nc.gpsimd.dma_start(out=nb_t[i], in_=nb[128 * i:128 * (i + 1), :])
                nc.gpsimd.dma_start(out=ob_t[i], in_=ob[128 * i:128 * (i + 1), :])

            kq = persist.tile([128, SEQ], BF16, tag="K", name="K")
            qq = persist.tile([128, HALF], BF16, tag="Q", name="Q")
            vt_b = persist.tile([128, SEQ], FP16, tag="VT", name="VT")
            qkvb = [persist.tile([128, 1], F32, tag=f"qkvb{m}", name=f"qkvb{m}") for m in (0, 2)]
            qkvb = {0: qkvb[0], 2: qkvb[1]}
            dp = [persist.tile([128, HALF], FP16, tag=f"dp{i}", name=f"dp{i}") for i in range(2)]
            nc.gpsimd.memset(dp[0], 0.0)
            nc.gpsimd.memset(dp[1], 0.0)

            # ---------------- GroupNorm statistics ----------------
            with tc.tile_pool(name="gn", bufs=1) as gn, \
                 tc.tile_pool(name="ps", bufs=2, space="PSUM") as ps:
                # dummy matmuls keep the PE out of its low p-state while the
                # x DMA + stats gate the real work
                dps = ps.tile([128, 2048], F32, tag="ps", name="ps")

                def warm(n):
                    for w in range(n):
                        nc.tensor.matmul(dps[0:1, 512 * (w % 2):512 * (w % 2 + 1)],
                                         ones_h, warm_t, start=True, stop=True,
                                         skip_group_check=True)
                warm(40)

                # bn_stats in chunk-arrival order
                stats = [gn.tile([128, 8, 6], F32, tag=f"st{i}", name=f"st{i}") for i in range(2)]
                stat_order = [(0, 0), (0, 1), (1, 0), (1, 1),
                              (0, 2), (0, 3), (1, 2), (1, 3),
                              (0, 4), (0, 5), (1, 4), (1, 5),
                              (0, 6), (0, 7), (1, 6), (1, 7)]
                for i, s in stat_order:
                    nc.vector.bn_stats(out=stats[i][:, s, :],
                                       in_=xkv[i][:, 512 * s:512 * (s + 1)])
                pp = [gn.tile([128, 2], F32, tag=f"pp{i}", name=f"pp{i}") for i in range(2)]
                for i in range(2):
                    mv = gn.tile([128, 2], F32, tag=f"mv{i}", name=f"mv{i}")
                    nc.vector.bn_aggr(out=mv, in_=stats[i])
                    # pp = (mean, E[x^2]) per partition
                    tmp = gn.tile([128, 1], F32, tag=f"tmp{i}", name=f"tmp{i}")
                    nc.vector.tensor_copy(pp[i][:, 0:1], mv[:, 0:1])
                    nc.vector.tensor_mul(tmp, mv[:, 0:1], mv[:, 0:1])
                    nc.vector.tensor_add(pp[i][:, 1:2], mv[:, 1:2], tmp)

                # group sums: psum[g, :] = sum over channels of group g
                gs_ps = ps.tile([128, 2048], F32, tag="ps", name="ps")
                for i in range(2):
                    nc.tensor.matmul(gs_ps[:, 0:2], gsel_t[i], pp[i],
                                     start=(i == 0), stop=(i == 1))
                # keep the PE warm across the stats-math window (in-order
                # queue: these run right after the gs matmuls)
                warm(22)
                gsb = gn.tile([128, 2], F32, tag="gsb", name="gsb")
                nc.vector.tensor_scalar_mul(gsb, gs_ps[:, 0:2], 1.0 / 32.0)
                varg = gn.tile([128, 1], F32, tag="varg", name="varg")
                tmp2 = gn.tile([128, 1], F32, tag="tmp2", name="tmp2")
                nc.vector.tensor_mul(tmp2, gsb[:, 0:1], gsb[:, 0:1])
                nc.vector.tensor_sub(varg, gsb[:, 1:2], tmp2)
                nc.vector.tensor_scalar_add(varg, varg, EPS)
                # rstd = rsqrt(varg): bit-hack seed + 2 Newton iterations (DVE
                # only -- keeps the Exp table resident on ScalarE)
                half_i = gn.tile([128, 1], I32, tag="halfi", name="halfi")
                y0b = gn.tile([128, 1], I32, tag="y0b", name="y0b")
                nc.vector.tensor_scalar(out=half_i, in0=varg.bitcast(I32),
                                        scalar1=1, scalar2=None,
                                        op0=OP.logical_shift_right)
                nc.vector.tensor_sub(y0b, magic_t, half_i)
                yk = y0b.bitcast(F32)
                rstd = gn.tile([128, 1], F32, tag="rstd", name="rstd")
                for it in range(2):
                    y2 = gn.tile([128, 1], F32, tag=f"y2_{it}", name=f"y2_{it}")
                    t_ = gn.tile([128, 1], F32, tag=f"t_{it}", name=f"t_{it}")
                    h_ = gn.tile([128, 1], F32, tag=f"h_{it}", name=f"h_{it}")
                    nxt = rstd if it == 1 else gn.tile([128, 1], F32, tag="y1", name="y1")
                    nc.vector.tensor_mul(y2, yk, yk)
                    nc.vector.tensor_mul(t_, varg, y2)
                    nc.vector.tensor_scalar(out=h_, in0=t_, scalar1=-0.5,
                                            scalar2=1.5, op0=OP.mult, op1=OP.add)
                    nc.vector.tensor_mul(nxt, yk, h_)
                    yk = nxt
                # gstats2 = (mean*rstd, rstd) per group-partition
                gstats = gn.tile([128, 2], F32, tag="gstats", name="gstats")
                nc.vector.tensor_mul(gstats[:, 0:1], gsb[:, 0:1], rstd)
                nc.vector.tensor_copy(gstats[:, 1:2], rstd)

                # broadcast to channels via nw-folded selector:
                # cs = (nw*mean*rstd, nw*rstd) ; a = cs1 ; b = nb - cs0
                for i in range(2):
                    cs_ps = ps.tile([128, 2048], F32, tag="ps", name="ps")
                    nc.tensor.matmul(cs_ps[:, 0:2], gselTn_t[:, 128 * i:128 * (i + 1)],
                                     gstats, start=True, stop=True)
                    nc.vector.tensor_copy(a_t[i], cs_ps[:, 1:2])
                    nc.vector.tensor_sub(b_t[i], nb_t[i], cs_ps[:, 0:1])

                # fold GroupNorm scale into QKV weights (bf16 out, one op)
                for i in range(2):
                    nc.vector.tensor_scalar(out=w2b[i], in0=wq_s[i],
                                            scalar1=a_t[i], scalar2=None,
                                            op0=OP.mult)
                # q bias (m=0) on the critical path; v bias (m=2) is only
                # needed at the first finalize and is emitted in the prologue.
                # k bias cancels in softmax.
                bp = ps.tile([128, 2048], F32, tag="ps", name="ps")
                for i in range(2):
                    nc.tensor.matmul(bp[:, 0:1], wq_s[i][:, 0:128],
                                     b_t[i], start=(i == 0), stop=(i == 1))
                nc.vector.tensor_copy(qkvb[0], bp[:, 0:1])

            # ---------------- attention ----------------
            with (
                tc.tile_pool(name="sgp", bufs=2, space="PSUM") as sgp,
                tc.tile_pool(name="accp", bufs=2, space="PSUM") as accp,
                tc.tile_pool(name="finp", bufs=2, space="PSUM") as finp,
                tc.tile_pool(name="apool", bufs=4) as apool,
                tc.tile_pool(name="upool", bufs=2) as upool,
                tc.tile_pool(name="fin", bufs=2) as fin,
            ):
                slots = [(c, t, p) for c in range(N_IC) for t in range(N_JT)
                         for p in range(2)]
                sg_of = {}
                acc_of = {}

                def emit_S(idx):
                    c, t, p = slots[idx]
                    sg = sgp.tile([128, 1024], F32, tag="sg", name="sg")
                    for hh in range(2):
                        h = 2 * p + hh
                        nc.tensor.matmul(
                            sg[:, 512 * hh:512 * (hh + 1)],
                            kq[32 * h:32 * (h + 1), 128 * t:128 * (t + 1)],
                            qq[32 * h:32 * (h + 1), 512 * c:512 * (c + 1)],
                            start=True, stop=True, tile_position=(32 * h, 0),
                        )
                    sg_of[idx] = sg

                def emit_qproj(icb, use_act):
                    qp = finp.tile([128, 512], F32, tag="fp", name="qp")
                    for i in range(2):
                        nc.tensor.matmul(qp, w2b[i][:, 0:HID],
                                         xq[i][:, 512 * icb:512 * (icb + 1)],
                                         start=(i == 0), stop=(i == 1))
                    dst = qq[:, 512 * icb:512 * (icb + 1)]
                    if use_act:
                        # ScalarE is idle pre-stream; Identity = in + bias
                        nc.scalar.activation(out=dst, in_=qp, func=AF.Identity,
                                             bias=qkvb[0], scale=1.0)
                    else:
                        nc.vector.tensor_scalar_add(dst, qp, qkvb[0])

                def emit_seg_K(seg):
                    # K' without bias (cancels in softmax); plain bf16 drain
                    sl = slice(512 * seg, 512 * (seg + 1))
                    kp = finp.tile([128, 512], F32, tag="fp", name="kp")
                    for i in range(2):
                        nc.tensor.matmul(kp, w2b[i][:, HID:2 * HID],
                                         xkv[i][:, sl], start=(i == 0), stop=(i == 1))
                    nc.vector.tensor_copy(kq[:, sl], kp)

                def emit_VT_group(g):
                    # V^T for j-tiles 4g..4g+3, one psum tile + one DVE drain
                    vtp = finp.tile([128, 512], F32, tag="fp", name="vtp")
                    for tt in range(4):
                        t = 4 * g + tt
                        for i in range(2):
                            nc.tensor.matmul(vtp[:, 128 * tt:128 * (tt + 1)],
                                             xkv[i][:, 128 * t:128 * (t + 1)],
                                             w2b[i][:, 2 * HID:3 * HID],
                                             start=(i == 0), stop=(i == 1))
                    nc.vector.tensor_copy(vt_b[:, 512 * g:512 * (g + 1)], vtp)

                fin_state = {}

                def finalize_d1(c):
                    # denominator matmuls, heads 0-1.  ones32 stationary
                    # broadcasts D_h over output rows 32h..32h+32, which
                    # row-aligns 1/D with o_acc's (h, d) rows -- no bsel
                    # broadcast matmul needed.
                    dcur = dp[c % 2]
                    d4 = finp.tile([128, 512], F32, tag="fp", name="d4")
                    for h in range(2):
                        nc.tensor.matmul(
                            d4[32 * h:32 * (h + 1), :], ones32,
                            dcur[:, 512 * h:512 * (h + 1)],
                            start=True, stop=True,
                            tile_position=(0, 32 * h), skip_group_check=True,
                        )
                    fin_state[c] = d4

                def finalize_d2(c):
                    # heads 2-3 + the reciprocal chain
                    dcur = dp[c % 2]
                    d4 = fin_state[c]
                    for h in range(2, NH):
                        nc.tensor.matmul(
                            d4[32 * h:32 * (h + 1), :], ones32,
                            dcur[:, 512 * h:512 * (h + 1)],
                            start=True, stop=True,
                            tile_position=(0, 32 * h), skip_group_check=True,
                        )
                    dmx = fin.tile([128, 512], F32, tag="dmx", name="dmx")
                    nc.vector.tensor_scalar_max(dmx, d4, 1e-30)
                    dr32 = fin.tile([128, 512], F32, tag="dr32", name="dr32")
                    scr = fin.tile([128, 512], F32, tag="scr", name="scr")
                    nc.vector.reciprocal_approx_accurate(out=dr32, in_=dmx,
                                                         scratch=scr)
                    drb = fin.tile([128, 512], BF16, tag="drb", name="drb")
                    nc.vector.tensor_copy(drb, dr32)
                    fin_state[c] = drb

                def finalize_piece(c, pc):
                    # normalize + out-proj + store for 256 queries; staggered
                    # across iterations to avoid a PE/DVE lump at chunk turns
                    drb = fin_state[c]
                    o_acc = acc_of[c]
                    w = 256
                    sl = slice(w * pc, w * (pc + 1))
                    o_sb = fin.tile([128, 256], F32, tag="osb", name="osb")
                    nc.vector.tensor_copy(o_sb, o_acc[:, sl])
                    on32 = fin.tile([128, 256], F32, tag="on32", name="on32")
                    on_b = fin.tile([128, 256], BF16, tag="onb", name="onb")
                    nc.vector.tensor_mul(on32, o_sb, drb[:, sl])
                    nc.vector.tensor_scalar_add(on_b, on32, qkvb[2])
                    for oc in range(2):
                        fo = finp.tile([128, 512], F32, tag="fp", name="fo")
                        nc.tensor.matmul(fo[:, 0:w],
                                         ow_b[:, 128 * oc:128 * (oc + 1)],
                                         on_b, start=True, stop=True)
                        ysb = fin.tile([128, 256], F32, tag="ysb", name="ysb")
                        nc.vector.tensor_scalar_add(ysb, fo[:, 0:w], ob_t[oc])
                        q_eng = nc.sync if oc == 0 else nc.gpsimd
                        q_eng.dma_start(
                            out=y[128 * oc:128 * (oc + 1),
                                  512 * c + w * pc:512 * c + w * (pc + 1)],
                            in_=ysb,
                        )

                def emit_PV(idx, a_t2):
                    c, t, p = slots[idx]
                    o_acc = acc_of[c]
                    last = (t == N_JT - 1 and p == 1)
                    for hh in range(2):
                        h = 2 * p + hh
                        nc.tensor.matmul(
                            o_acc[32 * h:32 * (h + 1), :],
                            vt_b[:, 128 * t + 32 * h:128 * t + 32 * (h + 1)],
                            a_t2[:, 512 * hh:512 * (hh + 1)],
                            start=(t == 0), stop=(last and hh == 1),
                            tile_position=(0, 32 * h), skip_group_check=True,
                        )

                # prologue: only seg0's K gates the first S/exp; VT group 0 is
                # needed first by PV(0), which runs after exp(0)
                emit_seg_K(0)
                emit_qproj(0, use_act=True)
                emit_S(0)
                emit_VT_group(0)
                # v bias (off critical path; first use at finalize of chunk 0)
                vbp = finp.tile([128, 512], F32, tag="fp", name="vbp")
                for i in range(2):
                    nc.tensor.matmul(vbp[:, 0:1], wq_s[i][:, 256:384],
                                     b_t[i], start=(i == 0), stop=(i == 1))
                nc.vector.tensor_copy(qkvb[2], vbp[:, 0:1])

                a_of = {}
                for idx, (c, t, p) in enumerate(slots):
                    if t == 0 and p == 0:
                        acc_of[c] = accp.tile([128, 512], F32, tag="Oacc", name="Oacc")

                    sg = sg_of.pop(idx)
                    a_t2 = apool.tile([128, 1024], FP16, tag="A", name="A")
                    a_of[idx] = a_t2
                    if sch_offload(c, t, p):
                        # DVE Schraudolph exp: u = A*S + B (fp16), then
                        # clamp+round to int16 whose BITS are the fp16 exp
                        u_t = upool.tile([128, 1024], FP16, tag="U", name="U")
                        nc.vector.tensor_scalar(out=u_t, in0=sg,
                                                scalar1=SCH_A, scalar2=SCH_B,
                                                op0=OP.mult, op1=OP.add)
                        nc.vector.tensor_scalar(out=a_t2.bitcast(mybir.dt.int16),
                                                in0=u_t,
                                                scalar1=32736.0, scalar2=0.0,
                                                op0=OP.min, op1=OP.max)
                    else:
                        nc.scalar.activation(out=a_t2, in_=sg, func=AF.Exp,
                                             scale=SCALE, bias=esh_t)
                    # S of the next half-slot goes on the PE queue BEFORE the
                    # delayed PV so the PE runs S(idx+1) first when exp(idx)
                    # completes -- ScalarE never waits on the PE.
                    if idx + 1 < len(slots):
                        emit_S(idx + 1)
                    if idx > 0:
                        emit_PV(idx - 1, a_of.pop(idx - 1))
                    if t == 0 and p == 0 and c > 0:
                        finalize_d2(c - 1)
                    # pieces spread deeper into the chunk: the y DMA has no
                    # deadline, and o_acc(c-1)'s buffer isn't rewritten until
                    # chunk c+1 -- this spreads the out-proj PE/DVE lump
                    if t == 2 and p == 0 and c > 0:
                        finalize_piece(c - 1, 0)
                    if t == 4 and p == 0 and c > 0:
                        finalize_piece(c - 1, 1)
                    if t == 0 and p == 0 and c + 1 < N_IC:
                        # after the delayed PV/finalize of chunk c-1 so the
                        # memset's WAR lands behind finalize's dp reads
                        nc.gpsimd.memset(dp[(c + 1) % 2], 0.0)
                    nc.vector.tensor_add(dp[c % 2][:, 1024 * p:1024 * (p + 1)],
                                         dp[c % 2][:, 1024 * p:1024 * (p + 1)],
                                         a_t2)
                    if t == N_JT - 1 and p == 1:
                        finalize_d1(c)
                    # projection spreading
                    if t == 8 and p == 0 and c + 1 < N_IC:
                        emit_qproj(c + 1, use_act=False)
                    if c == 0 and p == 1 and t % 4 == 1 and t // 4 + 1 < 8:
                        emit_seg_K(t // 4 + 1)
                    if c == 0 and p == 1 and t % 4 == 3 and t // 4 + 1 < 8:
                        emit_VT_group(t // 4 + 1)
                # tail: pipeline the last chunk's finalize in 256-col pieces
                # (o_sb copies and the D matmuls first, then per-piece
                # reciprocal chains interleaved with normalize/out-proj/DMA)
                cl = N_IC - 1
                emit_PV(n_last := len(slots) - 1, a_of.pop(n_last))
                dcur = dp[cl % 2]
                d4 = fin_state[cl]
                for h in range(2, NH):
                    nc.tensor.matmul(
                        d4[32 * h:32 * (h + 1), :], ones32,
                        dcur[:, 512 * h:512 * (h + 1)],
                        start=True, stop=True,
                        tile_position=(0, 32 * h), skip_group_check=True,
                    )
                o_sbs = []
                for pc in range(2):
                    o_sb = fin.tile([128, 256], F32, tag=f"osbt{pc}", name=f"osbt{pc}")
                    nc.vector.tensor_copy(o_sb, acc_of[cl][:, 256 * pc:256 * (pc + 1)])
                    o_sbs.append(o_sb)
                dmxs = []
                for pc in range(2):
                    # both d4 reads emitted before any finp reuse below
                    dmx = fin.tile([128, 256], F32, tag=f"dmxt{pc}", name=f"dmxt{pc}")
                    nc.vector.tensor_scalar_max(dmx, d4[:, 256 * pc:256 * (pc + 1)],
                                                1e-30)
                    dmxs.append(dmx)
                for pc in range(2):
                    dr32 = fin.tile([128, 256], F32, tag="dr32t", name="dr32t")
                    scr = fin.tile([128, 256], F32, tag="scrt", name="scrt")
                    nc.vector.reciprocal_approx_accurate(out=dr32, in_=dmxs[pc],
                                                         scratch=scr)
                    on32 = fin.tile([128, 256], F32, tag="on32", name="on32")
                    on_b = fin.tile([128, 256], BF16, tag="onb", name="onb")
                    nc.vector.tensor_mul(on32, o_sbs[pc], dr32)
                    nc.vector.tensor_scalar_add(on_b, on32, qkvb[2])
                    for oc in range(2):
                        fo = finp.tile([128, 512], F32, tag="fp", name="fo")
                        nc.tensor.matmul(fo[:, 0:256],
                                         ow_b[:, 128 * oc:128 * (oc + 1)],
                                         on_b, start=True, stop=True)
                        ysb = fin.tile([128, 256], F32, tag="ysb", name="ysb")
                        nc.vector.tensor_scalar_add(ysb, fo[:, 0:256], ob_t[oc])
                        q_eng = nc.sync if oc == 0 else nc.gpsimd
                        q_eng.dma_start(
                            out=y[128 * oc:128 * (oc + 1),
                                  512 * cl + 256 * pc:512 * cl + 256 * (pc + 1)],
                            in_=ysb,
                        )
    nc.compile()
    return nc


_NC_CACHE = {}


def _get_nc():
    if "nc" not in _NC_CACHE:
        _NC_CACHE["nc"] = build_program()
    return _NC_CACHE["nc"]


def _host_inputs(x, norm_w, norm_b, qkv_w, out_w, out_b):
    """Build the 8 per-core input maps."""
    import ml_dtypes
    x = np.asarray(x, dtype=np.float32)
    B = x.shape[0]
    xf = x.reshape(B, C, SEQ)
    xb = [np.ascontiguousarray(xf[b].astype(ml_dtypes.bfloat16)) for b in range(B)]

    wqkvT = np.ascontiguousarray(np.asarray(qkv_w, np.float32).T)      # [256, 384]
    owbT = np.ascontiguousarray(
        np.asarray(out_w, np.float32).T.astype(ml_dtypes.bfloat16))    # [128, 256]
    nw = np.asarray(norm_w, np.float32).reshape(C)
    nbv = np.asarray(norm_b, np.float32).reshape(C, 1).copy()
    obv = np.asarray(out_b, np.float32).reshape(C, 1).copy()

    gsel = np.zeros((C, 128), np.float32)
    for ch in range(C):
        gsel[ch, ch // 32] = 1.0
    # nw-folded transpose selector: gselTn[g, ch] = nw[ch] for ch in group g
    gselTn = np.zeros((128, C), np.float32)
    for ch in range(C):
        gselTn[ch // 32, ch] = nw[ch]

    in_maps = []
    for core in range(8):
        b, h = core // 2, core % 2
        in_maps.append({
            "x_kv": xb[b],
            "x_qb": np.ascontiguousarray(xb[b][:, HALF * h:HALF * (h + 1)]),
            "wqkvT": wqkvT, "owbT": owbT, "nb": nbv, "ob": obv,
            "gsel": gsel, "gselTn": gselTn,
        })
    return in_maps


def run(x, norm_w, norm_b, qkv_w, out_w, out_b, trace=False, tmpdir=None):
    """Run on 8 cores; returns (y_full, BassKernelResults)."""
    nc = _get_nc()
    in_maps = _host_inputs(x, norm_w, norm_b, qkv_w, out_w, out_b)
    res = run_bass_kernel_spmd(nc, in_maps, core_ids=list(range(8)), trace=trace,
                               tmpdir=tmpdir)
    x = np.asarray(x, dtype=np.float32)
    B = x.shape[0]
    HW_SIDE = int(np.sqrt(SEQ))
    out = np.empty((B, C, SEQ), np.float32)
    for core in range(8):
        b, h = core // 2, core % 2
        out[b][:, HALF * h:HALF * (h + 1)] = res.results[core]["y"]
    # exact fp32 residual added on host (kernel output excludes x)
    out += x.reshape(B, C, SEQ)
    return out.reshape(B, C, HW_SIDE, HW_SIDE), res


def kernel(x, norm_w, norm_b, qkv_w, out_w, out_b):
    y, _ = run(x, norm_w, norm_b, qkv_w, out_w, out_b, trace=False)
    return y


# revision 41
# speedup vs baseline: 1.2333x; 1.0025x over previous
"""Trainium2 Bass kernel for the SD-style spatial attention block:

    y = x + out_w @ attn(qkv(groupnorm(x))) + out_b    (per sample)

x: [4, 256, 64, 64] fp32.  GroupNorm(8 groups) -> 1x1 conv QKV (4 heads,
head_dim 32, seq = 64*64 = 4096) -> softmax attention -> 1x1 out conv + bias
+ residual.

Sharding over 8 NeuronCores: core c handles batch b = c//2 and query-half
h = c%2 (2048 of the 4096 query positions).  Each core receives the full
sample as bf16 (for GroupNorm stats and K/V over all positions) plus its
bf16 query slice, and produces y[b][:, 2048*h : 2048*(h+1)] WITHOUT the x
residual; the host adds the exact fp32 residual while gathering.

v11 changes over v10 (which was ScalarE-exp-bound at ~363us):
  - single bf16 copy of x on device; Q/K/V projections and GroupNorm stats
    all read it.  Input DMA drops to 3MB, x_kv prioritized on all 3 queues.
  - rstd via DVE Newton rsqrt (bit-hack seed + 2 iterations): no Sqrt
    activation -> the Exp table is loaded once at t=0 and never swapped.
  - K bias dropped entirely: a per-key additive S offset that is constant
    per softmax row cancels in A/D.  (Q bias kept; V bias folded in the
    finalize as before.)
  - V^T projection drains batched 4 tiles per DVE copy.
  - q-projection of chunk c+1 emitted inside chunk c (not chunk 0).
  - o_acc zeroed by start=True on each head's first PV matmul.
  - denominator broadcast by [128,32]-ones matmuls row-aligned to o_acc
    (no bsel matmul); finalize staggered across iterations in 256-col
    pieces; PE kept warm across the stats window by interleaved dummies.
"""
import sys

sys.path.insert(0, "/opt/trn_rl_repo")

import numpy as np

import concourse.bass as bass
import concourse.bacc as bacc
import concourse.tile as tile
from concourse import mybir
from concourse.bass_utils import run_bass_kernel_spmd

F32 = mybir.dt.float32
I32 = mybir.dt.int32
BF16 = mybir.dt.bfloat16
FP16 = mybir.dt.float16
AF = mybir.ActivationFunctionType
OP = mybir.AluOpType

C = 256          # input channels
HID = 128        # qkv hidden (4 heads x 32)
NH = 4
HD = 32
SEQ = 4096       # 64*64 spatial positions
HALF = 2048      # query positions per core
G = 8            # groups
EPS = 1e-5
SCALE = float(HD) ** -0.5
ESHIFT = -2.0    # constant exp shift; cancels in O/D normalization

N_IC = HALF // 512   # i-chunks per core (4)
N_JT = SEQ // 128    # j-tiles (32)
RSQRT_MAGIC = 0x5f3759df

# Schraudolph fp16 exp on the DVE: bitcast(int16(round(u))) ~= exp(s) for
# u = 1024*log2(e)*s + 15*1024 + C, s = SCALE*S + ESHIFT.  Validated on the
# real data: all-tile approx gives 1.9e-3 end-to-end rel err; we offload
# only a fraction of tiles to shave the ScalarE-bound stream pace.
LOG2E = 1.4426950408889634
SCH_A = SCALE * LOG2E * 1024.0
SCH_B = ESHIFT * LOG2E * 1024.0 + 15.0 * 1024.0 - 44.5


def sch_offload(c, t, p):
    # DISABLED: measured on HW, offloading whole tiles to the DVE stalls
    # the 2-deep S PSUM ring (the DVE holds the bank ~2us vs ScalarE's
    # 1.13us) and costs ~2.3us per offloaded tile.  Correctness was fine
    # (rel err 8e-4 at 1/8 offload).  Would need a 3-deep S ring.
    return False


def build_program():
    nc = bacc.Bacc()

    x_kv = nc.declare_dram_parameter("x_kv", [C, SEQ], BF16, isOutput=False)
    x_qb = nc.declare_dram_parameter("x_qb", [C, HALF], BF16, isOutput=False)
    wqkvT = nc.declare_dram_parameter("wqkvT", [C, 3 * HID], F32, isOutput=False)
    owbT = nc.declare_dram_parameter("owbT", [HID, C], BF16, isOutput=False)
    nb = nc.declare_dram_parameter("nb", [C, 1], F32, isOutput=False)
    ob = nc.declare_dram_parameter("ob", [C, 1], F32, isOutput=False)
    gsel = nc.declare_dram_parameter("gsel", [C, 128], F32, isOutput=False)
    gselTn = nc.declare_dram_parameter("gselTn", [128, C], F32, isOutput=False)
    y = nc.declare_dram_parameter("y", [C, HALF], BF16, isOutput=True)

    with tile.TileContext(nc) as tc:
        import contextlib
        with contextlib.ExitStack() as ctx:
            persist = ctx.enter_context(tc.tile_pool(name="persist", bufs=1))

            # ---------------- persistent tiles ----------------
            wq_s = [persist.tile([128, 3 * HID], F32, tag=f"wqs{i}", name=f"wqs{i}") for i in range(2)]
            w2b = [persist.tile([128, 3 * HID], BF16, tag=f"w2b{i}", name=f"w2b{i}") for i in range(2)]
            ow_b = persist.tile([128, C], BF16, tag="owb", name="owb")
            gsel_t = [persist.tile([128, 128], F32, tag=f"gsel{i}", name=f"gsel{i}") for i in range(2)]
            gselTn_t = persist.tile([128, C], F32, tag="gselTn", name="gselTn")
            nb_t = [persist.tile([128, 1], F32, tag=f"nb{i}", name=f"nb{i}") for i in range(2)]
            ob_t = [persist.tile([128, 1], F32, tag=f"ob{i}", name=f"ob{i}") for i in range(2)]
            ones_h = persist.tile([128, 1], FP16, tag="ones", name="ones")
            ones32 = persist.tile([128, 32], FP16, tag="ones32", name="ones32")
            eps_t = persist.tile([128, 1], F32, tag="eps", name="eps")
            esh_t = persist.tile([128, 1], F32, tag="esh", name="esh")
            magic_t = persist.tile([128, 1], I32, tag="magic", name="magic")
            warm_t = persist.tile([128, 512], FP16, tag="warm", name="warm")
            a_t = [persist.tile([128, 1], F32, tag=f"a{i}", name=f"a{i}") for i in range(2)]
            b_t = [persist.tile([128, 1], F32, tag=f"b{i}", name=f"b{i}") for i in range(2)]
            nc.vector.memset(ones_h, 1.0)
            nc.vector.memset(ones32, 1.0)
            nc.vector.memset(eps_t, EPS)
            nc.vector.memset(esh_t, ESHIFT)
            nc.vector.memset(warm_t, 0.0)
            nc.gpsimd.memset(magic_t, RSQRT_MAGIC)

            # pin the Exp activation table NOW (only table this kernel uses;
            # all later activations are Exp/Identity which share a set)
            scrA = persist.tile([128, 1], F32, tag="scrA", name="scrA")
            nc.scalar.activation(out=scrA, in_=eps_t, func=AF.Exp)

            # ---------------- input DMA ----------------
            xkv = [persist.tile([128, SEQ], BF16, tag=f"xkv{i}", name=f"xkv{i}") for i in range(2)]
            xq = [persist.tile([128, HALF], BF16, tag=f"xq{i}", name=f"xq{i}") for i in range(2)]
            CH = 1024
            chunk_q = [
                ((0, 0), nc.sync), ((1, 0), nc.gpsimd), ((0, 1), nc.scalar),
                ((1, 1), nc.sync), ((0, 2), nc.gpsimd), ((1, 2), nc.scalar),
                ((0, 3), nc.sync), ((1, 3), nc.gpsimd),
            ]
            for (i, p), q in chunk_q:
                q.dma_start(out=xkv[i][:, CH * p:CH * (p + 1)],
                            in_=x_kv[128 * i:128 * (i + 1), CH * p:CH * (p + 1)])
            # query-half x: first 512 cols early (gates qproj0), rest last
            nc.sync.dma_start(out=xq[0][:, 0:512], in_=x_qb[0:128, 0:512])
            nc.scalar.dma_start(out=xq[1][:, 0:512], in_=x_qb[128:256, 0:512])
            # weights/consts ride behind x
            nc.scalar.dma_start(out=wq_s[0], in_=wqkvT[0:128, :])
            nc.scalar.dma_start(out=wq_s[1], in_=wqkvT[128:256, :])
            nc.sync.dma_start(out=ow_b, in_=owbT[:, :])
            nc.gpsimd.dma_start(out=gselTn_t, in_=gselTn[:, :])
            for i in range(2):
                nc.sync.dma_start(out=gsel_t[i], in_=gsel[128 * i:128 * (i + 1), :])
                nc.gpsimd.dma_start(out=nb_t[i], in_=nb[128 * i:128 * (i + 1), :])
                nc.gpsimd.dma_start(out=ob_t[i], in_=ob[128 * i:128 * (i + 1), :])
            # xq tail: not needed until qproj(c+1) at ~45us+
            for i in range(2):
                nc.gpsimd.dma_start(out=xq[i][:, 512:HALF],
                                    in_=x_qb[128 * i:128 * (i + 1), 512:HALF])

            kq = persist.tile([128, SEQ], BF16, tag="K", name="K")
            qq = persist.tile([128, HALF], BF16, tag="Q", name="Q")
            vt_b = persist.tile([128, SEQ], FP16, tag="VT", name="VT")
            qkvb = [persist.tile([128, 1], F32, tag=f"qkvb{m}", name=f"qkvb{m}") for m in (0, 2)]
            qkvb = {0: qkvb[0], 2: qkvb[1]}
            dp = [persist.tile([128, HALF], FP16, tag=f"dp{i}", name=f"dp{i}") for i in range(2)]
            nc.gpsimd.memset(dp[0], 0.0)
            nc.gpsimd.memset(dp[1], 0.0)

            # ---------------- GroupNorm statistics ----------------
            with tc.tile_pool(name="gn", bufs=1) as gn, \
                 tc.tile_pool(name="ps", bufs=2, space="PSUM") as ps:
                # dummy matmuls keep the PE out of its low p-state while the
                # x DMA + stats gate the real work
                dps = ps.tile([128, 2048], F32, tag="ps", name="ps")

                def warm(n):
                    for w in range(n):
                        nc.tensor.matmul(dps[0:1, 512 * (w % 2):512 * (w % 2 + 1)],
                                         ones_h, warm_t, start=True, stop=True,
                                         skip_group_check=True)
                warm(40)

                # bn_stats in chunk-arrival order ([128,512] slices; the
                # bn_stats free dim is HW-capped at 512)
                stats = [gn.tile([128, 8, 6], F32, tag=f"st{i}", name=f"st{i}") for i in range(2)]
                stat_order = [(0, 0), (0, 1), (1, 0), (1, 1),
                              (0, 2), (0, 3), (1, 2), (1, 3),
                              (0, 4), (0, 5), (1, 4), (1, 5),
                              (0, 6), (0, 7), (1, 6), (1, 7)]
                for i, s in stat_order:
                    nc.vector.bn_stats(out=stats[i][:, s, :],
                                       in_=xkv[i][:, 512 * s:512 * (s + 1)])
                pp = [gn.tile([128, 2], F32, tag=f"pp{i}", name=f"pp{i}") for i in range(2)]
                for i in range(2):
                    mv = gn.tile([128, 2], F32, tag=f"mv{i}", name=f"mv{i}")
                    nc.vector.bn_aggr(out=mv, in_=stats[i])
                    # pp = (mean, E[x^2]) per partition
                    tmp = gn.tile([128, 1], F32, tag=f"tmp{i}", name=f"tmp{i}")
                    nc.vector.tensor_copy(pp[i][:, 0:1], mv[:, 0:1])
                    nc.vector.tensor_mul(tmp, mv[:, 0:1], mv[:, 0:1])
                    nc.vector.tensor_add(pp[i][:, 1:2], mv[:, 1:2], tmp)

                # group sums: psum[g, :] = sum over channels of group g
                gs_ps = ps.tile([128, 2048], F32, tag="ps", name="ps")
                for i in range(2):
                    nc.tensor.matmul(gs_ps[:, 0:2], gsel_t[i], pp[i],
                                     start=(i == 0), stop=(i == 1))
                # keep the PE warm across the stats-math window (in-order
                # queue: these run right after the gs matmuls)
                warm(22)
                gsb = gn.tile([128, 2], F32, tag="gsb", name="gsb")
                nc.vector.tensor_scalar_mul(gsb, gs_ps[:, 0:2], 1.0 / 32.0)
                varg = gn.tile([128, 1], F32, tag="varg", name="varg")
                tmp2 = gn.tile([128, 1], F32, tag="tmp2", name="tmp2")
                nc.vector.tensor_mul(tmp2, gsb[:, 0:1], gsb[:, 0:1])
                nc.vector.tensor_sub(varg, gsb[:, 1:2], tmp2)
                nc.vector.tensor_scalar_add(varg, varg, EPS)
                # rstd = rsqrt(varg): bit-hack seed + 2 Newton iterations (DVE
                # only -- keeps the Exp table resident on ScalarE)
                half_i = gn.tile([128, 1], I32, tag="halfi", name="halfi")
                y0b = gn.tile([128, 1], I32, tag="y0b", name="y0b")
                nc.vector.tensor_scalar(out=half_i, in0=varg.bitcast(I32),
                                        scalar1=1, scalar2=None,
                                        op0=OP.logical_shift_right)
                nc.vector.tensor_sub(y0b, magic_t, half_i)
                yk = y0b.bitcast(F32)
                rstd = gn.tile([128, 1], F32, tag="rstd", name="rstd")
                for it in range(2):
                    y2 = gn.tile([128, 1], F32, tag=f"y2_{it}", name=f"y2_{it}")
                    t_ = gn.tile([128, 1], F32, tag=f"t_{it}", name=f"t_{it}")
                    h_ = gn.tile([128, 1], F32, tag=f"h_{it}", name=f"h_{it}")
                    nxt = rstd if it == 1 else gn.tile([128, 1], F32, tag="y1", name="y1")
                    nc.vector.tensor_mul(y2, yk, yk)
                    nc.vector.tensor_mul(t_, varg, y2)
                    nc.vector.tensor_scalar(out=h_, in0=t_, scalar1=-0.5,
                                            scalar2=1.5, op0=OP.mult, op1=OP.add)
                    nc.vector.tensor_mul(nxt, yk, h_)
                    yk = nxt
                # gstats2 = (mean*rstd, rstd) per group-partition
                gstats = gn.tile([128, 2], F32, tag="gstats", name="gstats")
                nc.vector.tensor_mul(gstats[:, 0:1], gsb[:, 0:1], rstd)
                nc.vector.tensor_copy(gstats[:, 1:2], rstd)

                # broadcast to channels via nw-folded selector:
                # cs = (nw*mean*rstd, nw*rstd) ; a = cs1 ; b = nb - cs0
                for i in range(2):
                    cs_ps = ps.tile([128, 2048], F32, tag="ps", name="ps")
                    nc.tensor.matmul(cs_ps[:, 0:2], gselTn_t[:, 128 * i:128 * (i + 1)],
                                     gstats, start=True, stop=True)
                    nc.vector.tensor_copy(a_t[i], cs_ps[:, 1:2])
                    nc.vector.tensor_sub(b_t[i], nb_t[i], cs_ps[:, 0:1])

                # fold GroupNorm scale into QKV weights (bf16 out); K
                # columns first -- they gate K seg0 -> S(0) -> first exp
                for i in range(2):
                    nc.vector.tensor_scalar(out=w2b[i][:, HID:2 * HID],
                                            in0=wq_s[i][:, HID:2 * HID],
                                            scalar1=a_t[i], scalar2=None,
                                            op0=OP.mult)
                for i in range(2):
                    nc.vector.tensor_scalar(out=w2b[i][:, 0:HID],
                                            in0=wq_s[i][:, 0:HID],
                                            scalar1=a_t[i], scalar2=None,
                                            op0=OP.mult)
                for i in range(2):
                    nc.vector.tensor_scalar(out=w2b[i][:, 2 * HID:3 * HID],
                                            in0=wq_s[i][:, 2 * HID:3 * HID],
                                            scalar1=a_t[i], scalar2=None,
                                            op0=OP.mult)
                # q bias (m=0) on the critical path; v bias (m=2) is only
                # needed at the first finalize and is emitted in the prologue.
                # k bias cancels in softmax.
                bp = ps.tile([128, 2048], F32, tag="ps", name="ps")
                for i in range(2):
                    nc.tensor.matmul(bp[:, 0:1], wq_s[i][:, 0:128],
                                     b_t[i], start=(i == 0), stop=(i == 1))
                nc.vector.tensor_copy(qkvb[0], bp[:, 0:1])

            # ---------------- attention ----------------
            with (
                tc.tile_pool(name="sgp", bufs=2, space="PSUM") as sgp,
                tc.tile_pool(name="accp", bufs=2, space="PSUM") as accp,
                tc.tile_pool(name="finp", bufs=2, space="PSUM") as finp,
                tc.tile_pool(name="apool", bufs=4) as apool,
                tc.tile_pool(name="upool", bufs=2) as upool,
                tc.tile_pool(name="fin", bufs=2) as fin,
            ):
                slots = [(c, t, p) for c in range(N_IC) for t in range(N_JT)
                         for p in range(2)]
                sg_of = {}
                acc_of = {}

                def emit_S(idx):
                    c, t, p = slots[idx]
                    sg = sgp.tile([128, 1024], F32, tag="sg", name="sg")
                    for hh in range(2):
                        h = 2 * p + hh
                        nc.tensor.matmul(
                            sg[:, 512 * hh:512 * (hh + 1)],
                            kq[32 * h:32 * (h + 1), 128 * t:128 * (t + 1)],
                            qq[32 * h:32 * (h + 1), 512 * c:512 * (c + 1)],
                            start=True, stop=True, tile_position=(32 * h, 0),
                        )
                    sg_of[idx] = sg

                def emit_qproj(icb, use_act):
                    qp = finp.tile([128, 512], F32, tag="fp", name="qp")
                    for i in range(2):
                        nc.tensor.matmul(qp, w2b[i][:, 0:HID],
                                         xq[i][:, 512 * icb:512 * (icb + 1)],
                                         start=(i == 0), stop=(i == 1))
                    dst = qq[:, 512 * icb:512 * (icb + 1)]
                    if use_act:
                        # ScalarE is idle pre-stream; Identity = in + bias
                        nc.scalar.activation(out=dst, in_=qp, func=AF.Identity,
                                             bias=qkvb[0], scale=1.0)
                    else:
                        nc.vector.tensor_scalar_add(dst, qp, qkvb[0])

                def emit_seg_K(seg):
                    # K' without bias (cancels in softmax); plain bf16 drain
                    sl = slice(512 * seg, 512 * (seg + 1))
                    kp = finp.tile([128, 512], F32, tag="fp", name="kp")
                    for i in range(2):
                        nc.tensor.matmul(kp, w2b[i][:, HID:2 * HID],
                                         xkv[i][:, sl], start=(i == 0), stop=(i == 1))
                    nc.vector.tensor_copy(kq[:, sl], kp)

                def emit_VT_group(g):
                    # V^T for j-tiles 4g..4g+3, one psum tile + one DVE drain
                    vtp = finp.tile([128, 512], F32, tag="fp", name="vtp")
                    for tt in range(4):
                        t = 4 * g + tt
                        for i in range(2):
                            nc.tensor.matmul(vtp[:, 128 * tt:128 * (tt + 1)],
                                             xkv[i][:, 128 * t:128 * (t + 1)],
                                             w2b[i][:, 2 * HID:3 * HID],
                                             start=(i == 0), stop=(i == 1))
                    nc.vector.tensor_copy(vt_b[:, 512 * g:512 * (g + 1)], vtp)

                fin_state = {}

                def finalize_d1(c):
                    # denominator matmuls, heads 0-1.  ones32 stationary
                    # broadcasts D_h over output rows 32h..32h+32, which
                    # row-aligns 1/D with o_acc's (h, d) rows -- no bsel
                    # broadcast matmul needed.
                    dcur = dp[c % 2]
                    d4 = finp.tile([128, 512], F32, tag="fp", name="d4")
                    for h in range(2):
                        nc.tensor.matmul(
                            d4[32 * h:32 * (h + 1), :], ones32,
                            dcur[:, 512 * h:512 * (h + 1)],
                            start=True, stop=True,
                            tile_position=(0, 32 * h), skip_group_check=True,
                        )
                    fin_state[c] = d4

                def finalize_d2(c):
                    # heads 2-3 + the reciprocal chain
                    dcur = dp[c % 2]
                    d4 = fin_state[c]
                    for h in range(2, NH):
                        nc.tensor.matmul(
                            d4[32 * h:32 * (h + 1), :], ones32,
                            dcur[:, 512 * h:512 * (h + 1)],
                            start=True, stop=True,
                            tile_position=(0, 32 * h), skip_group_check=True,
                        )
                    dmx = fin.tile([128, 512], F32, tag="dmx", name="dmx")
                    nc.vector.tensor_scalar_max(dmx, d4, 1e-30)
                    dr32 = fin.tile([128, 512], F32, tag="dr32", name="dr32")
                    scr = fin.tile([128, 512], F32, tag="scr", name="scr")
                    nc.vector.reciprocal_approx_accurate(out=dr32, in_=dmx,
                                                         scratch=scr)
                    drb = fin.tile([128, 512], BF16, tag="drb", name="drb")
                    nc.vector.tensor_copy(drb, dr32)
                    fin_state[c] = drb

                def finalize_piece(c, pc):
                    # normalize + out-proj + store for 256 queries; staggered
                    # across iterations to avoid a PE/DVE lump at chunk turns
                    drb = fin_state[c]
                    o_acc = acc_of[c]
                    w = 256
                    sl = slice(w * pc, w * (pc + 1))
                    o_sb = fin.tile([128, 256], F32, tag="osb", name="osb")
                    nc.vector.tensor_copy(o_sb, o_acc[:, sl])
                    on32 = fin.tile([128, 256], F32, tag="on32", name="on32")
                    on_b = fin.tile([128, 256], BF16, tag="onb", name="onb")
                    nc.vector.tensor_mul(on32, o_sb, drb[:, sl])
                    nc.vector.tensor_scalar_add(on_b, on32, qkvb[2])
                    for oc in range(2):
                        fo = finp.tile([128, 512], F32, tag="fp", name="fo")
                        nc.tensor.matmul(fo[:, 0:w],
                                         ow_b[:, 128 * oc:128 * (oc + 1)],
                                         on_b, start=True, stop=True)
                        ysb = fin.tile([128, 256], BF16, tag="ysb", name="ysb")
                        nc.vector.tensor_scalar_add(ysb, fo[:, 0:w], ob_t[oc])
                        q_eng = nc.sync if oc == 0 else nc.gpsimd
                        q_eng.dma_start(
                            out=y[128 * oc:128 * (oc + 1),
                                  512 * c + w * pc:512 * c + w * (pc + 1)],
                            in_=ysb,
                        )

                def emit_PV(idx, a_t2):
                    c, t, p = slots[idx]
                    o_acc = acc_of[c]
                    last = (t == N_JT - 1 and p == 1)
                    for hh in range(2):
                        h = 2 * p + hh
                        nc.tensor.matmul(
                            o_acc[32 * h:32 * (h + 1), :],
                            vt_b[:, 128 * t + 32 * h:128 * t + 32 * (h + 1)],
                            a_t2[:, 512 * hh:512 * (hh + 1)],
                            start=(t == 0), stop=(last and hh == 1),
                            tile_position=(0, 32 * h), skip_group_check=True,
                        )

                # prologue: only seg0's K gates the first S/exp; VT group 0 is
                # needed first by PV(0), which runs after exp(0)
                emit_seg_K(0)
                emit_qproj(0, use_act=True)
                emit_S(0)
                emit_VT_group(0)
                # v bias (off critical path; first use at finalize of chunk 0)
                vbp = finp.tile([128, 512], F32, tag="fp", name="vbp")
                for i in range(2):
                    nc.tensor.matmul(vbp[:, 0:1], wq_s[i][:, 256:384],
                                     b_t[i], start=(i == 0), stop=(i == 1))
                nc.vector.tensor_copy(qkvb[2], vbp[:, 0:1])

                a_of = {}
                for idx, (c, t, p) in enumerate(slots):
                    if t == 0 and p == 0:
                        acc_of[c] = accp.tile([128, 512], F32, tag="Oacc", name="Oacc")

                    sg = sg_of.pop(idx)
                    a_t2 = apool.tile([128, 1024], FP16, tag="A", name="A")
                    a_of[idx] = a_t2
                    if sch_offload(c, t, p):
                        # DVE Schraudolph exp: u = A*S + B (fp16), then
                        # clamp+round to int16 whose BITS are the fp16 exp
                        u_t = upool.tile([128, 1024], FP16, tag="U", name="U")
                        nc.vector.tensor_scalar(out=u_t, in0=sg,
                                                scalar1=SCH_A, scalar2=SCH_B,
                                                op0=OP.mult, op1=OP.add)
                        nc.vector.tensor_scalar(out=a_t2.bitcast(mybir.dt.int16),
                                                in0=u_t,
                                                scalar1=32736.0, scalar2=0.0,
                                                op0=OP.min, op1=OP.max)
                    else:
                        nc.scalar.activation(out=a_t2, in_=sg, func=AF.Exp,
                                             scale=SCALE, bias=esh_t)
                    # S of the next half-slot goes on the PE queue BEFORE the
                    # delayed PV so the PE runs S(idx+1) first when exp(idx)
                    # completes -- ScalarE never waits on the PE.
                    if idx + 1 < len(slots):
                        emit_S(idx + 1)
                    if idx > 0:
                        emit_PV(idx - 1, a_of.pop(idx - 1))
                    if t == 0 and p == 0 and c > 0:
                        finalize_d2(c - 1)
                    # pieces spread deeper into the chunk: the y DMA has no
                    # deadline, and o_acc(c-1)'s buffer isn't rewritten until
                    # chunk c+1 -- this spreads the out-proj PE/DVE lump
                    if t == 2 and p == 0 and c > 0:
                        finalize_piece(c - 1, 0)
                    if t == 4 and p == 0 and c > 0:
                        finalize_piece(c - 1, 1)
                    if t == 0 and p == 0 and c + 1 < N_IC:
                        # after the delayed PV/finalize of chunk c-1 so the
                        # memset's WAR lands behind finalize's dp reads
                        nc.gpsimd.memset(dp[(c + 1) % 2], 0.0)
                    nc.vector.tensor_add(dp[c % 2][:, 1024 * p:1024 * (p + 1)],
                                         dp[c % 2][:, 1024 * p:1024 * (p + 1)],
                                         a_t2)
                    if t == N_JT - 1 and p == 1:
                        finalize_d1(c)
                    # projection spreading
                    if t == 8 and p == 0 and c + 1 < N_IC:
                        emit_qproj(c + 1, use_act=False)
                    if c == 0 and p == 1 and t % 4 == 1 and t // 4 + 1 < 8:
                        emit_seg_K(t // 4 + 1)
                    if c == 0 and p == 1 and t % 4 == 3 and t // 4 + 1 < 8:
                        emit_VT_group(t // 4 + 1)
                # tail: pipeline the last chunk's finalize in 256-col pieces
                # (o_sb copies and the D matmuls first, then per-piece
                # reciprocal chains interleaved with normalize/out-proj/DMA)
                cl = N_IC - 1
                emit_PV(n_last := len(slots) - 1, a_of.pop(n_last))
                dcur = dp[cl % 2]
                d4 = fin_state[cl]
                for h in range(2, NH):
                    nc.tensor.matmul(
                        d4[32 * h:32 * (h + 1), :], ones32,
                        dcur[:, 512 * h:512 * (h + 1)],
                        start=True, stop=True,
                        tile_position=(0, 32 * h), skip_group_check=True,
                    )
                o_sbs = []
                for pc in range(2):
                    o_sb = fin.tile([128, 256], F32, tag=f"osbt{pc}", name=f"osbt{pc}")
                    nc.vector.tensor_copy(o_sb, acc_of[cl][:, 256 * pc:256 * (pc + 1)])
                    o_sbs.append(o_sb)
                dmxs = []
                for pc in range(2):
                    # both d4 reads emitted before any finp reuse below
                    dmx = fin.tile([128, 256], F32, tag=f"dmxt{pc}", name=f"dmxt{pc}")
                    nc.vector.tensor_scalar_max(dmx, d4[:, 256 * pc:256 * (pc + 1)],
                                                1e-30)
                    dmxs.append(dmx)
                for pc in range(2):
                    dr32 = fin.tile([128, 256], F32, tag="dr32t", name="dr32t")
                    scr = fin.tile([128, 256], F32, tag="scrt", name="scrt")
                    nc.vector.reciprocal_approx_accurate(out=dr32, in_=dmxs[pc],
                                                         scratch=scr)
                    on32 = fin.tile([128, 256], F32, tag="on32", name="on32")
                    on_b = fin.tile([128, 256], BF16, tag="onb", name="onb")
                    nc.vector.tensor_mul(on32, o_sbs[pc], dr32)
                    nc.vector.tensor_scalar_add(on_b, on32, qkvb[2])
                    for oc in range(2):
                        fo = finp.tile([128, 512], F32, tag="fp", name="fo")
                        nc.tensor.matmul(fo[:, 0:256],
                                         ow_b[:, 128 * oc:128 * (oc + 1)],
                                         on_b, start=True, stop=True)
                        ysb = fin.tile([128, 256], BF16, tag="ysb", name="ysb")
                        nc.vector.tensor_scalar_add(ysb, fo[:, 0:256], ob_t[oc])
                        q_eng = nc.sync if oc == 0 else nc.gpsimd
                        q_eng.dma_start(
                            out=y[128 * oc:128 * (oc + 1),
                                  512 * cl + 256 * pc:512 * cl + 256 * (pc + 1)],
                            in_=ysb,
                        )
    nc.compile()
    return nc


_NC_CACHE = {}


def _get_nc():
    if "nc" not in _NC_CACHE:
        _NC_CACHE["nc"] = build_program()
    return _NC_CACHE["nc"]


def _host_inputs(x, norm_w, norm_b, qkv_w, out_w, out_b):
    """Build the 8 per-core input maps."""
    import ml_dtypes
    x = np.asarray(x, dtype=np.float32)
    B = x.shape[0]
    xf = x.reshape(B, C, SEQ)
    xb = [np.ascontiguousarray(xf[b].astype(ml_dtypes.bfloat16)) for b in range(B)]

    wqkvT = np.ascontiguousarray(np.asarray(qkv_w, np.float32).T)      # [256, 384]
    owbT = np.ascontiguousarray(
        np.asarray(out_w, np.float32).T.astype(ml_dtypes.bfloat16))    # [128, 256]
    nw = np.asarray(norm_w, np.float32).reshape(C)
    nbv = np.asarray(norm_b, np.float32).reshape(C, 1).copy()
    obv = np.asarray(out_b, np.float32).reshape(C, 1).copy()

    gsel = np.zeros((C, 128), np.float32)
    for ch in range(C):
        gsel[ch, ch // 32] = 1.0
    # nw-folded transpose selector: gselTn[g, ch] = nw[ch] for ch in group g
    gselTn = np.zeros((128, C), np.float32)
    for ch in range(C):
        gselTn[ch // 32, ch] = nw[ch]

    in_maps = []
    for core in range(8):
        b, h = core // 2, core % 2
        in_maps.append({
            "x_kv": xb[b],
            "x_qb": np.ascontiguousarray(xb[b][:, HALF * h:HALF * (h + 1)]),
            "wqkvT": wqkvT, "owbT": owbT, "nb": nbv, "ob": obv,
            "gsel": gsel, "gselTn": gselTn,
        })
    return in_maps


def run(x, norm_w, norm_b, qkv_w, out_w, out_b, trace=False, tmpdir=None):
    """Run on 8 cores; returns (y_full, BassKernelResults)."""
    nc = _get_nc()
    in_maps = _host_inputs(x, norm_w, norm_b, qkv_w, out_w, out_b)
    res = run_bass_kernel_spmd(nc, in_maps, core_ids=list(range(8)), trace=trace,
                               tmpdir=tmpdir)
    x = np.asarray(x, dtype=np.float32)
    B = x.shape[0]
    HW_SIDE = int(np.sqrt(SEQ))
    out = np.empty((B, C, SEQ), np.float32)
    for core in range(8):
        b, h = core // 2, core % 2
        out[b][:, HALF * h:HALF * (h + 1)] = np.asarray(
            res.results[core]["y"], dtype=np.float32)
    # exact fp32 residual added on host (kernel output excludes x)
    out += x.reshape(B, C, SEQ)
    return out.reshape(B, C, HW_SIDE, HW_SIDE), res


def kernel(x, norm_w, norm_b, qkv_w, out_w, out_b):
    y, _ = run(x, norm_w, norm_b, qkv_w, out_w, out_b, trace=False)
    return y


# revision 42
# speedup vs baseline: 1.2374x; 1.0033x over previous
"""Trainium2 Bass kernel for the SD-style spatial attention block:

    y = x + out_w @ attn(qkv(groupnorm(x))) + out_b    (per sample)

x: [4, 256, 64, 64] fp32.  GroupNorm(8 groups) -> 1x1 conv QKV (4 heads,
head_dim 32, seq = 64*64 = 4096) -> softmax attention -> 1x1 out conv + bias
+ residual.

Sharding over 8 NeuronCores: core c handles batch b = c//2 and query-half
h = c%2 (2048 of the 4096 query positions).  Each core receives the full
sample as bf16 (for GroupNorm stats and K/V over all positions) plus its
bf16 query slice, and produces y[b][:, 2048*h : 2048*(h+1)] WITHOUT the x
residual; the host adds the exact fp32 residual while gathering.

v11 changes over v10 (which was ScalarE-exp-bound at ~363us):
  - single bf16 copy of x on device; Q/K/V projections and GroupNorm stats
    all read it.  Input DMA drops to 3MB, x_kv prioritized on all 3 queues.
  - rstd via DVE Newton rsqrt (bit-hack seed + 2 iterations): no Sqrt
    activation -> the Exp table is loaded once at t=0 and never swapped.
  - K bias dropped entirely: a per-key additive S offset that is constant
    per softmax row cancels in A/D.  (Q bias kept; V bias folded in the
    finalize as before.)
  - V^T projection drains batched 4 tiles per DVE copy.
  - q-projection of chunk c+1 emitted inside chunk c (not chunk 0).
  - o_acc zeroed by start=True on each head's first PV matmul.
  - denominator broadcast by [128,32]-ones matmuls row-aligned to o_acc
    (no bsel matmul); finalize staggered across iterations in 256-col
    pieces; PE kept warm across the stats window by interleaved dummies.
"""
import sys

sys.path.insert(0, "/opt/trn_rl_repo")

import numpy as np

import concourse.bass as bass
import concourse.bacc as bacc
import concourse.tile as tile
from concourse import mybir
from concourse.bass_utils import run_bass_kernel_spmd

F32 = mybir.dt.float32
I32 = mybir.dt.int32
BF16 = mybir.dt.bfloat16
FP16 = mybir.dt.float16
AF = mybir.ActivationFunctionType
OP = mybir.AluOpType

C = 256          # input channels
HID = 128        # qkv hidden (4 heads x 32)
NH = 4
HD = 32
SEQ = 4096       # 64*64 spatial positions
HALF = 2048      # query positions per core
G = 8            # groups
EPS = 1e-5
SCALE = float(HD) ** -0.5
ESHIFT = -2.0    # constant exp shift; cancels in O/D normalization

N_IC = HALF // 512   # i-chunks per core (4)
N_JT = SEQ // 128    # j-tiles (32)
RSQRT_MAGIC = 0x5f3759df

# Schraudolph fp16 exp on the DVE: bitcast(int16(round(u))) ~= exp(s) for
# u = 1024*log2(e)*s + 15*1024 + C, s = SCALE*S + ESHIFT.  Validated on the
# real data: all-tile approx gives 1.9e-3 end-to-end rel err; we offload
# only a fraction of tiles to shave the ScalarE-bound stream pace.
LOG2E = 1.4426950408889634
SCH_A = SCALE * LOG2E * 1024.0
SCH_B = ESHIFT * LOG2E * 1024.0 + 15.0 * 1024.0 - 44.5


def sch_offload(c, t, p):
    # DISABLED: measured on HW, offloading whole tiles to the DVE stalls
    # the 2-deep S PSUM ring (the DVE holds the bank ~2us vs ScalarE's
    # 1.13us) and costs ~2.3us per offloaded tile.  Correctness was fine
    # (rel err 8e-4 at 1/8 offload).  Would need a 3-deep S ring.
    return False


def build_program():
    nc = bacc.Bacc()

    x_kv = nc.declare_dram_parameter("x_kv", [C, SEQ], BF16, isOutput=False)
    x_qb = nc.declare_dram_parameter("x_qb", [C, HALF], BF16, isOutput=False)
    wqkvT = nc.declare_dram_parameter("wqkvT", [C, 3 * HID], F32, isOutput=False)
    owbT = nc.declare_dram_parameter("owbT", [HID, C], BF16, isOutput=False)
    nb = nc.declare_dram_parameter("nb", [C, 1], F32, isOutput=False)
    ob = nc.declare_dram_parameter("ob", [C, 1], F32, isOutput=False)
    gsel = nc.declare_dram_parameter("gsel", [C, 128], F32, isOutput=False)
    gselTn = nc.declare_dram_parameter("gselTn", [128, C], F32, isOutput=False)
    y = nc.declare_dram_parameter("y", [C, HALF], BF16, isOutput=True)

    with tile.TileContext(nc) as tc:
        import contextlib
        with contextlib.ExitStack() as ctx:
            persist = ctx.enter_context(tc.tile_pool(name="persist", bufs=1))

            # ---------------- persistent tiles ----------------
            wq_s = [persist.tile([128, 3 * HID], F32, tag=f"wqs{i}", name=f"wqs{i}") for i in range(2)]
            w2b = [persist.tile([128, 3 * HID], BF16, tag=f"w2b{i}", name=f"w2b{i}") for i in range(2)]
            ow_b = persist.tile([128, C], BF16, tag="owb", name="owb")
            gsel_t = [persist.tile([128, 128], F32, tag=f"gsel{i}", name=f"gsel{i}") for i in range(2)]
            gselTn_t = persist.tile([128, C], F32, tag="gselTn", name="gselTn")
            nb_t = [persist.tile([128, 1], F32, tag=f"nb{i}", name=f"nb{i}") for i in range(2)]
            ob_t = [persist.tile([128, 1], F32, tag=f"ob{i}", name=f"ob{i}") for i in range(2)]
            ones_h = persist.tile([128, 1], FP16, tag="ones", name="ones")
            ones32 = persist.tile([128, 32], FP16, tag="ones32", name="ones32")
            eps_t = persist.tile([128, 1], F32, tag="eps", name="eps")
            esh_t = persist.tile([128, 1], F32, tag="esh", name="esh")
            magic_t = persist.tile([128, 1], I32, tag="magic", name="magic")
            warm_t = persist.tile([128, 512], FP16, tag="warm", name="warm")
            a_t = [persist.tile([128, 1], F32, tag=f"a{i}", name=f"a{i}") for i in range(2)]
            b_t = [persist.tile([128, 1], F32, tag=f"b{i}", name=f"b{i}") for i in range(2)]
            nc.vector.memset(ones_h, 1.0)
            nc.vector.memset(ones32, 1.0)
            nc.vector.memset(eps_t, EPS)
            nc.vector.memset(esh_t, ESHIFT)
            nc.vector.memset(warm_t, 0.0)
            nc.gpsimd.memset(magic_t, RSQRT_MAGIC)

            # pin the Exp activation table NOW (only table this kernel uses;
            # all later activations are Exp/Identity which share a set)
            scrA = persist.tile([128, 1], F32, tag="scrA", name="scrA")
            nc.scalar.activation(out=scrA, in_=eps_t, func=AF.Exp)

            # ---------------- input DMA ----------------
            xkv = [persist.tile([128, SEQ], BF16, tag=f"xkv{i}", name=f"xkv{i}") for i in range(2)]
            xq = [persist.tile([128, HALF], BF16, tag=f"xq{i}", name=f"xq{i}") for i in range(2)]
            qs = [nc.sync, nc.gpsimd, nc.scalar]
            seq16 = [(i, sfx) for sfx in range(8) for i in range(2)]
            for n, (i, sfx) in enumerate(seq16):
                qs[n % 3].dma_start(
                    out=xkv[i][:, 512 * sfx:512 * (sfx + 1)],
                    in_=x_kv[128 * i:128 * (i + 1), 512 * sfx:512 * (sfx + 1)])
            # query-half x: first 512 cols early (gates qproj0), rest last
            nc.sync.dma_start(out=xq[0][:, 0:512], in_=x_qb[0:128, 0:512])
            nc.scalar.dma_start(out=xq[1][:, 0:512], in_=x_qb[128:256, 0:512])
            # weights/consts ride behind x
            nc.scalar.dma_start(out=wq_s[0], in_=wqkvT[0:128, :])
            nc.scalar.dma_start(out=wq_s[1], in_=wqkvT[128:256, :])
            nc.sync.dma_start(out=ow_b, in_=owbT[:, :])
            nc.gpsimd.dma_start(out=gselTn_t, in_=gselTn[:, :])
            for i in range(2):
                nc.sync.dma_start(out=gsel_t[i], in_=gsel[128 * i:128 * (i + 1), :])
                nc.gpsimd.dma_start(out=nb_t[i], in_=nb[128 * i:128 * (i + 1), :])
                nc.gpsimd.dma_start(out=ob_t[i], in_=ob[128 * i:128 * (i + 1), :])
            # xq tail: not needed until qproj(c+1) at ~45us+
            for i in range(2):
                nc.gpsimd.dma_start(out=xq[i][:, 512:HALF],
                                    in_=x_qb[128 * i:128 * (i + 1), 512:HALF])

            kq = persist.tile([128, SEQ], BF16, tag="K", name="K")
            qq = persist.tile([128, HALF], BF16, tag="Q", name="Q")
            vt_b = persist.tile([128, SEQ], FP16, tag="VT", name="VT")
            qkvb = [persist.tile([128, 1], F32, tag=f"qkvb{m}", name=f"qkvb{m}") for m in (0, 2)]
            qkvb = {0: qkvb[0], 2: qkvb[1]}
            dp = [persist.tile([128, HALF], FP16, tag=f"dp{i}", name=f"dp{i}") for i in range(2)]
            nc.gpsimd.memset(dp[0], 0.0)
            nc.gpsimd.memset(dp[1], 0.0)

            # ---------------- GroupNorm statistics ----------------
            with tc.tile_pool(name="gn", bufs=1) as gn, \
                 tc.tile_pool(name="ps", bufs=2, space="PSUM") as ps:
                # dummy matmuls keep the PE out of its low p-state while the
                # x DMA + stats gate the real work
                dps = ps.tile([128, 2048], F32, tag="ps", name="ps")

                def warm(n):
                    for w in range(n):
                        nc.tensor.matmul(dps[0:1, 512 * (w % 2):512 * (w % 2 + 1)],
                                         ones_h, warm_t, start=True, stop=True,
                                         skip_group_check=True)
                warm(40)

                # bn_stats in chunk-arrival order ([128,512] slices; the
                # bn_stats free dim is HW-capped at 512)
                stats = [gn.tile([128, 8, 6], F32, tag=f"st{i}", name=f"st{i}") for i in range(2)]
                stat_order = [(i, sfx) for sfx in range(8) for i in range(2)]
                for i, s in stat_order:
                    nc.vector.bn_stats(out=stats[i][:, s, :],
                                       in_=xkv[i][:, 512 * s:512 * (s + 1)])
                pp = []
                for i in range(2):
                    mv = gn.tile([128, 2], F32, tag=f"mv{i}", name=f"mv{i}")
                    nc.vector.bn_aggr(out=mv, in_=stats[i])
                    # mv[:,1] <- E[x^2] = var + mean^2 in place
                    tmp = gn.tile([128, 1], F32, tag=f"tmp{i}", name=f"tmp{i}")
                    nc.vector.tensor_mul(tmp, mv[:, 0:1], mv[:, 0:1])
                    nc.vector.tensor_add(mv[:, 1:2], mv[:, 1:2], tmp)
                    pp.append(mv)

                # group sums: psum[g, :] = sum over channels of group g
                gs_ps = ps.tile([128, 2048], F32, tag="ps", name="ps")
                for i in range(2):
                    nc.tensor.matmul(gs_ps[:, 0:2], gsel_t[i], pp[i],
                                     start=(i == 0), stop=(i == 1))
                # keep the PE warm across the stats-math window (in-order
                # queue: these run right after the gs matmuls)
                warm(22)
                gsb = gn.tile([128, 2], F32, tag="gsb", name="gsb")
                nc.vector.tensor_scalar_mul(gsb, gs_ps[:, 0:2], 1.0 / 32.0)
                varg = gn.tile([128, 1], F32, tag="varg", name="varg")
                tmp2 = gn.tile([128, 1], F32, tag="tmp2", name="tmp2")
                nc.vector.tensor_mul(tmp2, gsb[:, 0:1], gsb[:, 0:1])
                # varg = (E[x^2]g + EPS) - mean^2 in one fused op
                nc.vector.scalar_tensor_tensor(
                    out=varg, in0=gsb[:, 1:2], scalar=EPS, in1=tmp2,
                    op0=OP.add, op1=OP.subtract)
                # rstd = rsqrt(varg): bit-hack seed + 2 Newton iterations (DVE
                # only -- keeps the Exp table resident on ScalarE)
                half_i = gn.tile([128, 1], I32, tag="halfi", name="halfi")
                y0b = gn.tile([128, 1], I32, tag="y0b", name="y0b")
                nc.vector.tensor_scalar(out=half_i, in0=varg.bitcast(I32),
                                        scalar1=1, scalar2=None,
                                        op0=OP.logical_shift_right)
                nc.vector.tensor_sub(y0b, magic_t, half_i)
                yk = y0b.bitcast(F32)
                rstd = gn.tile([128, 1], F32, tag="rstd", name="rstd")
                for it in range(2):
                    y2 = gn.tile([128, 1], F32, tag=f"y2_{it}", name=f"y2_{it}")
                    t_ = gn.tile([128, 1], F32, tag=f"t_{it}", name=f"t_{it}")
                    h_ = gn.tile([128, 1], F32, tag=f"h_{it}", name=f"h_{it}")
                    nxt = rstd if it == 1 else gn.tile([128, 1], F32, tag="y1", name="y1")
                    nc.vector.tensor_mul(y2, yk, yk)
                    nc.vector.tensor_mul(t_, varg, y2)
                    nc.vector.tensor_scalar(out=h_, in0=t_, scalar1=-0.5,
                                            scalar2=1.5, op0=OP.mult, op1=OP.add)
                    nc.vector.tensor_mul(nxt, yk, h_)
                    yk = nxt
                # gstats2 = (mean*rstd, rstd) per group-partition
                gstats = gn.tile([128, 2], F32, tag="gstats", name="gstats")
                nc.vector.tensor_mul(gstats[:, 0:1], gsb[:, 0:1], rstd)
                nc.vector.tensor_copy(gstats[:, 1:2], rstd)

                # broadcast to channels via nw-folded selector:
                # cs = (nw*mean*rstd, nw*rstd) ; a = cs1 ; b = nb - cs0
                for i in range(2):
                    cs_ps = ps.tile([128, 2048], F32, tag="ps", name="ps")
                    nc.tensor.matmul(cs_ps[:, 0:2], gselTn_t[:, 128 * i:128 * (i + 1)],
                                     gstats, start=True, stop=True)
                    nc.vector.tensor_copy(a_t[i], cs_ps[:, 1:2])
                    nc.vector.tensor_sub(b_t[i], nb_t[i], cs_ps[:, 0:1])

                # fold GroupNorm scale into QKV weights (bf16 out); K
                # columns first -- they gate K seg0 -> S(0) -> first exp
                for i in range(2):
                    nc.vector.tensor_scalar(out=w2b[i][:, HID:2 * HID],
                                            in0=wq_s[i][:, HID:2 * HID],
                                            scalar1=a_t[i], scalar2=None,
                                            op0=OP.mult)
                for i in range(2):
                    nc.vector.tensor_scalar(out=w2b[i][:, 0:HID],
                                            in0=wq_s[i][:, 0:HID],
                                            scalar1=a_t[i], scalar2=None,
                                            op0=OP.mult)
                for i in range(2):
                    nc.vector.tensor_scalar(out=w2b[i][:, 2 * HID:3 * HID],
                                            in0=wq_s[i][:, 2 * HID:3 * HID],
                                            scalar1=a_t[i], scalar2=None,
                                            op0=OP.mult)
                # q bias (m=0) on the critical path; v bias (m=2) is only
                # needed at the first finalize and is emitted in the prologue.
                # k bias cancels in softmax.
                bp = ps.tile([128, 2048], F32, tag="ps", name="ps")
                for i in range(2):
                    nc.tensor.matmul(bp[:, 0:1], wq_s[i][:, 0:128],
                                     b_t[i], start=(i == 0), stop=(i == 1))
                nc.vector.tensor_copy(qkvb[0], bp[:, 0:1])

            # ---------------- attention ----------------
            with (
                tc.tile_pool(name="sgp", bufs=2, space="PSUM") as sgp,
                tc.tile_pool(name="accp", bufs=2, space="PSUM") as accp,
                tc.tile_pool(name="finp", bufs=2, space="PSUM") as finp,
                tc.tile_pool(name="apool", bufs=4) as apool,
                tc.tile_pool(name="upool", bufs=2) as upool,
                tc.tile_pool(name="fin", bufs=2) as fin,
            ):
                slots = [(c, t, p) for c in range(N_IC) for t in range(N_JT)
                         for p in range(2)]
                sg_of = {}
                acc_of = {}

                def emit_S(idx):
                    c, t, p = slots[idx]
                    sg = sgp.tile([128, 1024], F32, tag="sg", name="sg")
                    for hh in range(2):
                        h = 2 * p + hh
                        nc.tensor.matmul(
                            sg[:, 512 * hh:512 * (hh + 1)],
                            kq[32 * h:32 * (h + 1), 128 * t:128 * (t + 1)],
                            qq[32 * h:32 * (h + 1), 512 * c:512 * (c + 1)],
                            start=True, stop=True, tile_position=(32 * h, 0),
                        )
                    sg_of[idx] = sg

                def emit_qproj(icb, use_act):
                    qp = finp.tile([128, 512], F32, tag="fp", name="qp")
                    for i in range(2):
                        nc.tensor.matmul(qp, w2b[i][:, 0:HID],
                                         xq[i][:, 512 * icb:512 * (icb + 1)],
                                         start=(i == 0), stop=(i == 1))
                    dst = qq[:, 512 * icb:512 * (icb + 1)]
                    if use_act:
                        # ScalarE is idle pre-stream; Identity = in + bias
                        nc.scalar.activation(out=dst, in_=qp, func=AF.Identity,
                                             bias=qkvb[0], scale=1.0)
                    else:
                        nc.vector.tensor_scalar_add(dst, qp, qkvb[0])

                def emit_seg_K(seg):
                    # K' without bias (cancels in softmax); plain bf16 drain
                    sl = slice(512 * seg, 512 * (seg + 1))
                    kp = finp.tile([128, 512], F32, tag="fp", name="kp")
                    for i in range(2):
                        nc.tensor.matmul(kp, w2b[i][:, HID:2 * HID],
                                         xkv[i][:, sl], start=(i == 0), stop=(i == 1))
                    nc.vector.tensor_copy(kq[:, sl], kp)

                def emit_VT_group(g):
                    # V^T for j-tiles 4g..4g+3, one psum tile + one DVE drain
                    vtp = finp.tile([128, 512], F32, tag="fp", name="vtp")
                    for tt in range(4):
                        t = 4 * g + tt
                        for i in range(2):
                            nc.tensor.matmul(vtp[:, 128 * tt:128 * (tt + 1)],
                                             xkv[i][:, 128 * t:128 * (t + 1)],
                                             w2b[i][:, 2 * HID:3 * HID],
                                             start=(i == 0), stop=(i == 1))
                    nc.vector.tensor_copy(vt_b[:, 512 * g:512 * (g + 1)], vtp)

                fin_state = {}

                def finalize_d1(c):
                    # denominator matmuls, heads 0-1.  ones32 stationary
                    # broadcasts D_h over output rows 32h..32h+32, which
                    # row-aligns 1/D with o_acc's (h, d) rows -- no bsel
                    # broadcast matmul needed.
                    dcur = dp[c % 2]
                    d4 = finp.tile([128, 512], F32, tag="fp", name="d4")
                    for h in range(2):
                        nc.tensor.matmul(
                            d4[32 * h:32 * (h + 1), :], ones32,
                            dcur[:, 512 * h:512 * (h + 1)],
                            start=True, stop=True,
                            tile_position=(0, 32 * h), skip_group_check=True,
                        )
                    fin_state[c] = d4

                def finalize_d2(c):
                    # heads 2-3 + the reciprocal chain
                    dcur = dp[c % 2]
                    d4 = fin_state[c]
                    for h in range(2, NH):
                        nc.tensor.matmul(
                            d4[32 * h:32 * (h + 1), :], ones32,
                            dcur[:, 512 * h:512 * (h + 1)],
                            start=True, stop=True,
                            tile_position=(0, 32 * h), skip_group_check=True,
                        )
                    dmx = fin.tile([128, 512], F32, tag="dmx", name="dmx")
                    nc.vector.tensor_scalar_max(dmx, d4, 1e-30)
                    dr32 = fin.tile([128, 512], F32, tag="dr32", name="dr32")
                    scr = fin.tile([128, 512], F32, tag="scr", name="scr")
                    nc.vector.reciprocal_approx_accurate(out=dr32, in_=dmx,
                                                         scratch=scr)
                    drb = fin.tile([128, 512], BF16, tag="drb", name="drb")
                    nc.vector.tensor_copy(drb, dr32)
                    fin_state[c] = drb

                def finalize_piece(c, pc):
                    # normalize + out-proj + store for 256 queries; staggered
                    # across iterations to avoid a PE/DVE lump at chunk turns
                    drb = fin_state[c]
                    o_acc = acc_of[c]
                    w = 256
                    sl = slice(w * pc, w * (pc + 1))
                    o_sb = fin.tile([128, 256], F32, tag="osb", name="osb")
                    nc.vector.tensor_copy(o_sb, o_acc[:, sl])
                    on32 = fin.tile([128, 256], F32, tag="on32", name="on32")
                    on_b = fin.tile([128, 256], BF16, tag="onb", name="onb")
                    nc.vector.tensor_mul(on32, o_sb, drb[:, sl])
                    nc.vector.tensor_scalar_add(on_b, on32, qkvb[2])
                    for oc in range(2):
                        fo = finp.tile([128, 512], F32, tag="fp", name="fo")
                        nc.tensor.matmul(fo[:, 0:w],
                                         ow_b[:, 128 * oc:128 * (oc + 1)],
                                         on_b, start=True, stop=True)
                        ysb = fin.tile([128, 256], BF16, tag="ysb", name="ysb")
                        nc.vector.tensor_scalar_add(ysb, fo[:, 0:w], ob_t[oc])
                        q_eng = nc.sync if oc == 0 else nc.gpsimd
                        q_eng.dma_start(
                            out=y[128 * oc:128 * (oc + 1),
                                  512 * c + w * pc:512 * c + w * (pc + 1)],
                            in_=ysb,
                        )

                def emit_PV(idx, a_t2):
                    c, t, p = slots[idx]
                    o_acc = acc_of[c]
                    last = (t == N_JT - 1 and p == 1)
                    for hh in range(2):
                        h = 2 * p + hh
                        nc.tensor.matmul(
                            o_acc[32 * h:32 * (h + 1), :],
                            vt_b[:, 128 * t + 32 * h:128 * t + 32 * (h + 1)],
                            a_t2[:, 512 * hh:512 * (hh + 1)],
                            start=(t == 0), stop=(last and hh == 1),
                            tile_position=(0, 32 * h), skip_group_check=True,
                        )

                # prologue: only j-tile 0 of K gates the first S/exp; the
                # rest of seg0 and VT group 0 follow (PV(0) runs after exp(0))
                kp0 = finp.tile([128, 512], F32, tag="fp", name="kp0")
                for i in range(2):
                    nc.tensor.matmul(kp0[:, 0:128], w2b[i][:, HID:2 * HID],
                                     xkv[i][:, 0:128], start=(i == 0), stop=(i == 1))
                nc.vector.tensor_copy(kq[:, 0:128], kp0[:, 0:128])
                emit_qproj(0, use_act=True)
                kp0b = finp.tile([128, 512], F32, tag="fp", name="kp0b")
                for i in range(2):
                    nc.tensor.matmul(kp0b[:, 0:384], w2b[i][:, HID:2 * HID],
                                     xkv[i][:, 128:512], start=(i == 0), stop=(i == 1))
                emit_S(0)
                nc.vector.tensor_copy(kq[:, 128:512], kp0b[:, 0:384])
                emit_VT_group(0)
                # v bias (off critical path; first use at finalize of chunk 0)
                vbp = finp.tile([128, 512], F32, tag="fp", name="vbp")
                for i in range(2):
                    nc.tensor.matmul(vbp[:, 0:1], wq_s[i][:, 256:384],
                                     b_t[i], start=(i == 0), stop=(i == 1))
                nc.vector.tensor_copy(qkvb[2], vbp[:, 0:1])

                a_of = {}
                for idx, (c, t, p) in enumerate(slots):
                    if t == 0 and p == 0:
                        acc_of[c] = accp.tile([128, 512], F32, tag="Oacc", name="Oacc")

                    sg = sg_of.pop(idx)
                    a_t2 = apool.tile([128, 1024], FP16, tag="A", name="A")
                    a_of[idx] = a_t2
                    if sch_offload(c, t, p):
                        # DVE Schraudolph exp: u = A*S + B (fp16), then
                        # clamp+round to int16 whose BITS are the fp16 exp
                        u_t = upool.tile([128, 1024], FP16, tag="U", name="U")
                        nc.vector.tensor_scalar(out=u_t, in0=sg,
                                                scalar1=SCH_A, scalar2=SCH_B,
                                                op0=OP.mult, op1=OP.add)
                        nc.vector.tensor_scalar(out=a_t2.bitcast(mybir.dt.int16),
                                                in0=u_t,
                                                scalar1=32736.0, scalar2=0.0,
                                                op0=OP.min, op1=OP.max)
                    else:
                        nc.scalar.activation(out=a_t2, in_=sg, func=AF.Exp,
                                             scale=SCALE, bias=esh_t)
                    # S of the next half-slot goes on the PE queue BEFORE the
                    # delayed PV so the PE runs S(idx+1) first when exp(idx)
                    # completes -- ScalarE never waits on the PE.
                    if idx + 1 < len(slots):
                        emit_S(idx + 1)
                    if idx > 0:
                        emit_PV(idx - 1, a_of.pop(idx - 1))
                    if t == 0 and p == 0 and c > 0:
                        finalize_d2(c - 1)
                    # pieces spread deeper into the chunk: the y DMA has no
                    # deadline, and o_acc(c-1)'s buffer isn't rewritten until
                    # chunk c+1 -- this spreads the out-proj PE/DVE lump
                    if t == 2 and p == 0 and c > 0:
                        finalize_piece(c - 1, 0)
                    if t == 4 and p == 0 and c > 0:
                        finalize_piece(c - 1, 1)
                    if t == 0 and p == 0 and c + 1 < N_IC:
                        # after the delayed PV/finalize of chunk c-1 so the
                        # memset's WAR lands behind finalize's dp reads
                        nc.gpsimd.memset(dp[(c + 1) % 2], 0.0)
                    nc.vector.tensor_add(dp[c % 2][:, 1024 * p:1024 * (p + 1)],
                                         dp[c % 2][:, 1024 * p:1024 * (p + 1)],
                                         a_t2)
                    if t == N_JT - 1 and p == 1:
                        finalize_d1(c)
                    # projection spreading
                    if t == 8 and p == 0 and c + 1 < N_IC:
                        emit_qproj(c + 1, use_act=False)
                    if c == 0 and p == 1 and t % 4 == 1 and t // 4 + 1 < 8:
                        emit_seg_K(t // 4 + 1)
                    if c == 0 and p == 1 and t % 4 == 3 and t // 4 + 1 < 8:
                        emit_VT_group(t // 4 + 1)
                # tail: pipeline the last chunk's finalize in 256-col pieces
                # (o_sb copies and the D matmuls first, then per-piece
                # reciprocal chains interleaved with normalize/out-proj/DMA)
                cl = N_IC - 1
                emit_PV(n_last := len(slots) - 1, a_of.pop(n_last))
                dcur = dp[cl % 2]
                d4 = fin_state[cl]
                for h in range(2, NH):
                    nc.tensor.matmul(
                        d4[32 * h:32 * (h + 1), :], ones32,
                        dcur[:, 512 * h:512 * (h + 1)],
                        start=True, stop=True,
                        tile_position=(0, 32 * h), skip_group_check=True,
                    )
                o_sbs = []
                for pc in range(2):
                    o_sb = fin.tile([128, 256], F32, tag=f"osbt{pc}", name=f"osbt{pc}")
                    nc.vector.tensor_copy(o_sb, acc_of[cl][:, 256 * pc:256 * (pc + 1)])
                    o_sbs.append(o_sb)
                dmxs = []
                for pc in range(2):
                    # both d4 reads emitted before any finp reuse below
                    dmx = fin.tile([128, 256], F32, tag=f"dmxt{pc}", name=f"dmxt{pc}")
                    nc.vector.tensor_scalar_max(dmx, d4[:, 256 * pc:256 * (pc + 1)],
                                                1e-30)
                    dmxs.append(dmx)
                for pc in range(2):
                    dr32 = fin.tile([128, 256], F32, tag="dr32t", name="dr32t")
                    scr = fin.tile([128, 256], F32, tag="scrt", name="scrt")
                    nc.vector.reciprocal_approx_accurate(out=dr32, in_=dmxs[pc],
                                                         scratch=scr)
                    on32 = fin.tile([128, 256], F32, tag="on32", name="on32")
                    on_b = fin.tile([128, 256], BF16, tag="onb", name="onb")
                    nc.vector.tensor_mul(on32, o_sbs[pc], dr32)
                    nc.vector.tensor_scalar_add(on_b, on32, qkvb[2])
                    for oc in range(2):
                        fo = finp.tile([128, 512], F32, tag="fp", name="fo")
                        nc.tensor.matmul(fo[:, 0:256],
                                         ow_b[:, 128 * oc:128 * (oc + 1)],
                                         on_b, start=True, stop=True)
                        ysb = fin.tile([128, 256], BF16, tag="ysb", name="ysb")
                        nc.vector.tensor_scalar_add(ysb, fo[:, 0:256], ob_t[oc])
                        q_eng = nc.sync if oc == 0 else nc.gpsimd
                        q_eng.dma_start(
                            out=y[128 * oc:128 * (oc + 1),
                                  512 * cl + 256 * pc:512 * cl + 256 * (pc + 1)],
                            in_=ysb,
                        )
    nc.compile()
    return nc


_NC_CACHE = {}


def _get_nc():
    if "nc" not in _NC_CACHE:
        _NC_CACHE["nc"] = build_program()
    return _NC_CACHE["nc"]


def _host_inputs(x, norm_w, norm_b, qkv_w, out_w, out_b):
    """Build the 8 per-core input maps."""
    import ml_dtypes
    x = np.asarray(x, dtype=np.float32)
    B = x.shape[0]
    xf = x.reshape(B, C, SEQ)
    xb = [np.ascontiguousarray(xf[b].astype(ml_dtypes.bfloat16)) for b in range(B)]

    wqkvT = np.ascontiguousarray(np.asarray(qkv_w, np.float32).T)      # [256, 384]
    owbT = np.ascontiguousarray(
        np.asarray(out_w, np.float32).T.astype(ml_dtypes.bfloat16))    # [128, 256]
    nw = np.asarray(norm_w, np.float32).reshape(C)
    nbv = np.asarray(norm_b, np.float32).reshape(C, 1).copy()
    obv = np.asarray(out_b, np.float32).reshape(C, 1).copy()

    gsel = np.zeros((C, 128), np.float32)
    for ch in range(C):
        gsel[ch, ch // 32] = 1.0
    # nw-folded transpose selector: gselTn[g, ch] = nw[ch] for ch in group g
    gselTn = np.zeros((128, C), np.float32)
    for ch in range(C):
        gselTn[ch // 32, ch] = nw[ch]

    in_maps = []
    for core in range(8):
        b, h = core // 2, core % 2
        in_maps.append({
            "x_kv": xb[b],
            "x_qb": np.ascontiguousarray(xb[b][:, HALF * h:HALF * (h + 1)]),
            "wqkvT": wqkvT, "owbT": owbT, "nb": nbv, "ob": obv,
            "gsel": gsel, "gselTn": gselTn,
        })
    return in_maps


def run(x, norm_w, norm_b, qkv_w, out_w, out_b, trace=False, tmpdir=None):
    """Run on 8 cores; returns (y_full, BassKernelResults)."""
    nc = _get_nc()
    in_maps = _host_inputs(x, norm_w, norm_b, qkv_w, out_w, out_b)
    res = run_bass_kernel_spmd(nc, in_maps, core_ids=list(range(8)), trace=trace,
                               tmpdir=tmpdir)
    x = np.asarray(x, dtype=np.float32)
    B = x.shape[0]
    HW_SIDE = int(np.sqrt(SEQ))
    out = np.empty((B, C, SEQ), np.float32)
    for core in range(8):
        b, h = core // 2, core % 2
        out[b][:, HALF * h:HALF * (h + 1)] = np.asarray(
            res.results[core]["y"], dtype=np.float32)
    # exact fp32 residual added on host (kernel output excludes x)
    out += x.reshape(B, C, SEQ)
    return out.reshape(B, C, HW_SIDE, HW_SIDE), res


def kernel(x, norm_w, norm_b, qkv_w, out_w, out_b):
    y, _ = run(x, norm_w, norm_b, qkv_w, out_w, out_b, trace=False)
    return y
